# revision 5
# baseline (speedup 1.0000x reference)
"""Attention-LSTM (CaptioningRNN) Trainium2 kernel, v2.

Data-parallel over N=128 across 8 cores (16 samples each), with every
per-step tensor kept TRANSPOSED (feature dim on the 128 partitions, the
16 local samples on the free dim) so each matmul's output free size is
16 and the gate GEMMs run weight-stationary:

  act^T[j, n] = sum_k Wh[k, j] h^T[k, n]          (256 matmuls, ap16)
              + sum_(n',m) AW2[(n',m), j] wdT[(n',m), n]   (64 matmuls)
              + xW^T[j, (t, n)]                   (1 identity matmul)

where AW2[(n,m), j] = sum_h A[n, h, m] Wattn[h, j] is precomputed once
(phase 1), which turns the whole attention-apply + attn@Wattn step into
a real 256-deep contraction: wdT[(n',m), n] = softmax_w[n, m] * (n'==n)
is built by a tiny scatter-matmul from the softmax weights.

Phase 1 also computes xW^T = (x @ Wx + b)^T into a DRAM scratch in a
layout that lets each step load its slice with large descriptors.

h/c state lives as [128 (h mod 128), 8 (h chunk), 16 (n)]; h^T is
written directly into the y-store ring buffer that both the next step's
matmuls and the (batched) y DMA read.
"""

import sys
from contextlib import ExitStack

sys.path.insert(0, "/opt/trn_rl_repo")

import numpy as np

import concourse.bass as bass  # noqa: F401
import concourse.mybir as mybir
import concourse.tile as tile
from concourse import bacc
from concourse.bass_utils import run_bass_kernel_spmd

F32 = mybir.dt.float32
BF16 = mybir.dt.float16  # IEEE fp16: same PE rate as bf16, more mantissa

N, T, D, H = 128, 64, 1024, 1024
K4 = 4 * H            # 4096
NCORES = 8
NL = N // NCORES      # 16 samples per core
M = 16                # spatial positions (4x4)
KC = H // 128         # 8 contraction chunks
JC = K4 // 128        # 32 output chunks
SCALE = 1.0 / float(np.sqrt(H))
TB = 4                # steps per xw-load / y-store block
N_WARM = 16
WARM_EN = False
GATE_MODE = 'four'
G_FIRST = True
EXP_ACCUM = True
GATE_COPY = False

Alu = mybir.AluOpType
Act = mybir.ActivationFunctionType

_cache = {}


def _build(steps=T):
    key = ("nc", steps)
    if key in _cache:
        return _cache[key]

    nc = bacc.Bacc("TRN2", target_bir_lowering=False)

    # ---- kernel I/O ----------------------------------------------------
    d_xT = nc.dram_tensor("xT", [D, T * NL], BF16, kind="ExternalInput")
    d_A = nc.dram_tensor("A", [NL, H, M], F32, kind="ExternalInput")
    d_Wx = nc.dram_tensor("Wx", [D, K4], BF16, kind="ExternalInput")
    d_Wh = nc.dram_tensor("Wh", [H, K4], BF16, kind="ExternalInput")
    d_Wa = nc.dram_tensor("Wa", [H, K4], BF16, kind="ExternalInput")
    d_bT = nc.dram_tensor("bT", [128, JC], F32, kind="ExternalInput")
    d_id = nc.dram_tensor("id128", [128, 128], BF16, kind="ExternalInput")
    d_sel = nc.dram_tensor("selc", [16, 2 * 128], BF16, kind="ExternalInput")
    d_mD = nc.dram_tensor("maskD", [128, 2 * 16], BF16, kind="ExternalInput")
    d_mdg = nc.dram_tensor("maskdiag", [16, 16], F32, kind="ExternalInput")
    # outputs / scratch
    d_y = nc.dram_tensor("y", [128, T, KC, NL], BF16, kind="ExternalOutput")
    d_xw = nc.dram_tensor("xw", [128, JC, T, NL], BF16)

    with tile.TileContext(nc) as tc:
      with (
          tc.tile_pool(name="wts", bufs=1) as wts,    # persistent weights
          tc.tile_pool(name="stt", bufs=1) as stt,    # persistent state
      ):
        # ---------------- persistent SBUF tensors -----------------------
        at4 = wts.tile([128, KC, NL, M], BF16, tag="at4")
        aw2 = wts.tile([128, 2, JC, 128], BF16, tag="aw2")
        id128 = wts.tile([128, 128], BF16, tag="id128")
        selc = wts.tile([16, 2, 128], BF16, tag="selc")
        maskD = wts.tile([128, 2, NL], BF16, tag="maskD")
        maskdg = wts.tile([16, 16], F32, tag="maskdg")
        bT = wts.tile([128, JC], F32, tag="bT")

        c_sb = stt.tile([128, KC, NL], F32, tag="c")
        hT0 = stt.tile([128, KC, NL], BF16, tag="hT0")
        # y ring: 2 buffers x TB steps; h^T of step t lives in
        # yring[t % 2TB]; matmuls of step t+1 read it, y DMA stores it.
        yring = stt.tile([128, 2 * TB, KC, NL], BF16, tag="yring")
        g_i = stt.tile([128, KC, NL], F32, tag="g_i")
        g_f = stt.tile([128, KC, NL], F32, tag="g_f")
        g_o = stt.tile([128, KC, NL], F32, tag="g_o")
        g_g = stt.tile([128, KC, NL], F32, tag="g_g")
        th = stt.tile([128, KC, NL], F32, tag="th")
        w16 = stt.tile([32, 32], BF16, tag="w16")
        w16T = stt.tile([32, 32], BF16, tag="w16T")
        sc16 = stt.tile([16, M], F32, tag="sc16")
        ex16 = stt.tile([16, 16], F32, tag="ex16")
        u16 = stt.tile([16, 16], F32, tag="u16")
        sm16 = stt.tile([16, 1], F32, tag="sm16")
        rc16 = stt.tile([16, 1], F32, tag="rc16")
        wdT = stt.tile([128, 2, NL], BF16, tag="wdT")
        xwr = stt.tile([128, 2, JC, TB, NL], BF16, tag="xwr")


        # ====== Phase 1: xW^T GEMM + AW GEMM, DMA-pipelined =============
        # DMA issue order = consumption order: xt, Wx j-slices (xW GEMM
        # starts after ~2 slices), A chunks, Wa j-slices (AW GEMM
        # pipelines), then Wh into Wa's freed space (not needed until
        # the first recurrent step, so its load hides under the GEMMs).
        _st = ExitStack()
        whp = _st.enter_context(tc.tile_pool(name="whp", bufs=1))
        wh = whp.tile([128, KC, K4], BF16, tag="wh")
        p1bw = _st.enter_context(tc.tile_pool(name="p1bw", bufs=1))
        p1wx = _st.enter_context(tc.tile_pool(name="p1wx", bufs=5))
        p1bs = _st.enter_context(tc.tile_pool(name="p1bs", bufs=14))
        p1a = _st.enter_context(tc.tile_pool(name="p1a", bufs=8))
        if True:
            xt = p1bw.tile([128, KC, T * NL], BF16, tag="xt")
            d_xT_r = d_xT.rearrange("(kc p) r -> p kc r", p=128)
            d_Wx_r = d_Wx.rearrange("(kc p) f -> p kc f", p=128)
            wxs = []

            def load_wx(jb):
                wxj = p1wx.tile([128, KC, 512], BF16, tag="wx")
                nc.sync.dma_start(out=wxj[:],
                                  in_=d_Wx_r[:, :, jb * 512:(jb + 1) * 512])
                wxs.append(wxj)

            nc.sync.dma_start(out=xt[:, :, 0:512], in_=d_xT_r[:, :, 0:512])
            load_wx(0)
            nc.sync.dma_start(out=xt[:, :, 512:1024],
                              in_=d_xT_r[:, :, 512:1024])
            for jb in range(1, 8):
                load_wx(jb)
            a_sts = []
            for kc in range(KC):
                a_st = p1a.tile([128, NL, M], F32, tag="a_st")
                nc.sync.dma_start(
                    out=a_st[:],
                    in_=d_A.rearrange("n (kc p) m -> kc p n m", p=128)[kc])
                a_sts.append(a_st)
            # small-const loads
            nc.sync.dma_start(out=id128[:], in_=d_id[:])
            nc.sync.dma_start(
                out=selc[:], in_=d_sel.rearrange("p (h q) -> p h q", h=2))
            nc.sync.dma_start(
                out=maskD[:], in_=d_mD.rearrange("p (h n) -> p h n", h=2))
            nc.sync.dma_start(out=maskdg[:], in_=d_mdg[:])
            nc.sync.dma_start(out=bT[:], in_=d_bT[:])
            nc.vector.memset(w16[:], 0.0)
            nc.vector.memset(w16T[:], 0.0)

            with (
                tc.tile_pool(name="p1w", bufs=3) as p1w,
                tc.tile_pool(name="p1bp", bufs=6, space="PSUM") as p1bp,
            ):
                # at4 (bf16) and c0 = mean_m A from the staged chunks
                for kc in range(KC):
                    nc.vector.tensor_copy(at4[:, kc], a_sts[kc][:])
                    nc.vector.tensor_reduce(
                        c_sb[:, kc, :], a_sts[kc][:],
                        axis=mybir.AxisListType.X, op=Alu.add)
                nc.vector.tensor_scalar_mul(hT0[:], c_sb[:], 1.0 / M)
                nc.scalar.mul(c_sb[:], c_sb[:], 1.0 / M)

                # xW^T GEMM first (chases the Wx slice DMAs), then the
                # AW GEMM (chases the Wa slices) — strictly in PE order.
                for jb in range(8):
                    for jc in range(4 * jb, 4 * jb + 4):
                        for tb in range(2):
                            pxw = p1bp.tile([128, 512], F32, tag="pxw")
                            for kc in range(KC):
                                nc.tensor.matmul(
                                    pxw[:],
                                    wxs[jb][:, kc,
                                            (jc - 4 * jb) * 128:
                                            (jc - 4 * jb + 1) * 128],
                                    xt[:, kc, tb * 512:(tb + 1) * 512],
                                    start=(kc == 0), stop=(kc == KC - 1))
                            ob = p1bs.tile([128, 512], BF16, tag="p1out")
                            nc.vector.tensor_scalar_add(
                                ob[:], pxw[:], bT[:, jc:jc + 1])
                            nc.sync.dma_start(
                                out=d_xw[:, jc, tb * 32:(tb + 1) * 32, :]
                                .rearrange("p t n -> p (t n)"),
                                in_=ob[:])
                d_Wa_r = d_Wa.rearrange("(kc p) f -> p kc f", p=128)
                was = []
                for jb in range(8):
                    waj = p1w.tile([128, KC, 512], BF16, tag="wa")
                    nc.sync.dma_start(
                        out=waj[:],
                        in_=d_Wa_r[:, :, jb * 512:(jb + 1) * 512])
                    was.append(waj)
                # Wh load (sliced so it interleaves with the ring
                # reloads on the serial DMA device) overlaps phase 1
                d_Wh_r = d_Wh.rearrange("(kc p) f -> p kc f", p=128)
                for jb in range(8):
                    nc.sync.dma_start(
                        out=wh[:, :, jb * 512:(jb + 1) * 512],
                        in_=d_Wh_r[:, :, jb * 512:(jb + 1) * 512])

                for jb in range(8):
                    for h in range(2):
                        paw = p1bp.tile([128, 512], F32, tag="pxw")
                        for kc in range(KC):
                            nc.tensor.matmul(
                                paw[:],
                                at4[:, kc, 8 * h:8 * h + 8, :].rearrange(
                                    "p n m -> p (n m)"),
                                was[jb][:, kc, :],
                                start=(kc == 0), stop=(kc == KC - 1))
                        nc.vector.tensor_copy(
                            aw2[:, h, 4 * jb:4 * jb + 4, :].rearrange(
                                "p jc q -> p (jc q)"),
                            paw[:])

        # ================== Phase 2: recurrent steps ====================
        actp = _st.enter_context(tc.tile_pool(name="actp", bufs=3, space="PSUM"))
        scp = _st.enter_context(tc.tile_pool(name="scp", bufs=1, space="PSUM"))
        bbp = _st.enter_context(tc.tile_pool(name="bbp", bufs=1, space="PSUM"))
        smls = _st.enter_context(tc.tile_pool(name="smls", bufs=2))
        if True:
            def warm_on(lhsT, rhs, k):
                if not WARM_EN:
                    return
                """Low-priority PE filler matmuls that become ready only
                once `lhsT` is written: they bridge PE idle gaps in the
                recurrent chain so the p-state ramp stays hot, yielding
                to any ready real matmul (which carries higher priority).
                """
                p = int(np.prod(lhsT.shape[1:]))
                f = int(np.prod(rhs.shape[1:]))
                for _ in range(k):
                    nc.tensor.matmul(
                        pdum[0:p, 0:f], lhsT, rhs,
                        start=True, stop=True)

            # prefetch xw for t-blocks 0 and 1
            for b in range(2):
                nc.sync.dma_start(
                    out=xwr[:, b], in_=d_xw[:, :, b * TB:(b + 1) * TB, :])

            # per-gate act psum tiles: consecutive ACT reads of one
            # psum tile serialize at +219ns each; separate tiles pipeline
            pact = [None, None, None]

            def alloc_bank():
                pa = actp.tile([128, 2, KC, NL], F32, tag="pactA")
                pb = actp.tile([128, 2, KC, NL], F32, tag="pactB")
                return (pa, pb)

            def cur_slice(bank, jc):
                # jc 0..31 -> gate g = jc//8: i,f in tile A; o,g in B
                g = jc // 8
                return bank[g // 2][:, g % 2, jc % 8, :]

            def xw_add(bank, tt):
                # one start=True matmul per psum tile (jc 0:16 -> A,
                # 16:32 -> B) so the accumulation group opens the whole
                # tile exactly once
                for half in range(2):
                    nc.tensor.matmul(
                        bank[half][:],
                        id128[:],
                        xwr[:, tt // TB % 2, 16 * half:16 * half + 16,
                            tt % TB, :],
                        start=True, stop=False,
                        skip_group_check=True)

            for tt in range(min(2, steps)):
                bank = alloc_bank()
                pact[tt % 3] = bank
                xw_add(bank, tt)

            if G_FIRST:
                JC_ORDER = list(range(24, 32)) + list(range(0, 24))
            else:
                JC_ORDER = list(range(0, 16)) + list(range(24, 32)) + \
                    list(range(16, 24))

            for t in range(steps):
                cur = pact[t % 3]
                hT = hT0 if t == 0 else yring[:, (t - 1) % (2 * TB)]

                # -- scores: S[n', (n, m)] accumulated over h chunks
                psc = scp.tile([16, NL * M], F32, tag="psc")
                for kc in range(KC):
                    nc.tensor.matmul(
                        psc[:], hT[:, kc, :],
                        at4[:, kc].rearrange("p n m -> p (n m)"),
                        start=(kc == 0), stop=(kc == KC - 1))

                # -- Wh gate matmuls (independent of attention chain)
                for jc in range(JC):
                    for kc in range(KC):
                        nc.tensor.matmul(
                            cur_slice(cur, jc),
                            wh[:, kc, jc * 128:(jc + 1) * 128],
                            hT[:, kc, :], start=False, stop=False,
                            skip_group_check=True)

                # -- softmax chain (DVE/ACT) off the scores
                smul = smls.tile([16, NL, M], F32, tag="smul")
                nc.vector.tensor_tensor(
                    smul[:],
                    psc[:].rearrange("p (n m) -> p n m", n=NL),
                    maskdg[:].unsqueeze(2).broadcast_to([16, NL, M]),
                    op=Alu.mult)
                nc.vector.tensor_reduce(
                    sc16[:], smul[:].rearrange("p n m -> p m n"),
                    axis=mybir.AxisListType.X, op=Alu.add)
                # exp(x) = 1/sigmoid(-x) - 1: keeps every activation in
                # the 'sigmoid_and_others' HW table (sigmoid+tanh), so no
                # 1283ns act-table reload is needed anywhere in the loop.
                # |score*scale| <= ~3, so sigmoid never saturates and the
                # r-1 cancellation only affects negligibly small weights.
                nc.scalar.activation(
                    ex16[:], sc16[:], Act.Sigmoid, scale=-SCALE)
                nc.vector.reciprocal(u16[:], ex16[:])
                nc.vector.tensor_scalar(
                    ex16[:], u16[:], -1.0, 0.0, op0=Alu.add, op1=Alu.add,
                    accum_out=sm16[:])
                nc.vector.reciprocal(rc16[:], sm16[:])
                nc.vector.tensor_scalar_mul(
                    w16[0:16, 0:16], ex16[:], rc16[:])
                nc.vector.transpose(w16T[:], w16[:])
                # bridge softmax-chain PE idle (after Wh drains)
                warm_on(ex16[:], wh[0:32, 0, 0:512], 4)

                # -- scatter w^T across 256 (n,m) partitions, mask diag
                pbb = bbp.tile([128, 2, NL], F32, tag="pbb")
                for h in range(2):
                    nc.tensor.matmul(
                        pbb[:, h, :], selc[:, h, :], w16T[0:16, 0:16],
                        start=True, stop=True)
                # xw identity-add for step t+1 fills the PE stall while
                # maskD runs on DVE
                if t + 2 < steps:
                    nxt = alloc_bank()
                    pact[(t + 2) % 3] = nxt
                    xw_add(nxt, t + 2)
                warm_on(w16T[:], wh[0:32, 0, 0:512], 3)
                nc.vector.tensor_tensor(
                    wdT[:], pbb[:], maskD[:], op=Alu.mult)

                # -- attention gate matmuls close each jc's psum group
                # (o-gate jc 16..23 last so its ACT is off the c-path)
                for jc in JC_ORDER:
                    for h in range(2):
                        nc.tensor.matmul(
                            cur_slice(cur, jc),
                            aw2[:, h, jc, :], wdT[:, h, :],
                            start=False, stop=(h == 1),
                            skip_group_check=True)

                # -- gate activations: tanh(g) first (its attn matmuls
                # close first), then ONE sigmoid over the contiguous
                # i/f/o block — each PSUM-reading ACT carries a ~219ns
                # pipeline penalty, so fewer, bigger ACTs win.
                gi, gf, go, gg = g_i, g_f, g_o, g_g
                # alternate tiles A/B so consecutive ACTs pipeline
                nc.scalar.activation(gi[:], cur[0][:, 0], Act.Sigmoid)
                nc.scalar.activation(gg[:], cur[1][:, 1], Act.Tanh)
                nc.scalar.activation(gf[:], cur[0][:, 1], Act.Sigmoid)
                nc.scalar.activation(go[:], cur[1][:, 0], Act.Sigmoid)

                # -- c = f*c + i*g ; h = o*tanh(c)
                fc = smls.tile([128, KC, NL], F32, tag="fc")
                nc.vector.tensor_tensor(fc[:], gf[:], c_sb[:], op=Alu.mult)
                ig = smls.tile([128, KC, NL], F32, tag="ig")
                nc.vector.tensor_tensor(ig[:], gi[:], gg[:], op=Alu.mult)
                nc.vector.tensor_tensor(c_sb[:], fc[:], ig[:], op=Alu.add)
                nc.scalar.activation(th[:], c_sb[:], Act.Tanh)
                hout = yring[:, t % (2 * TB)]
                for q in range(4):
                    nc.vector.tensor_tensor(
                        hout[:, 2 * q:2 * q + 2], go[:, 2 * q:2 * q + 2],
                        th[:, 2 * q:2 * q + 2], op=Alu.mult)

                # -- batched y store + xw prefetch every TB steps
                if t % TB == TB - 1:
                    blk = t // TB
                    nc.sync.dma_start(
                        out=d_y[:, blk * TB:(blk + 1) * TB, :, :],
                        in_=yring[:, (blk % 2) * TB:(blk % 2) * TB + TB])
                    if (blk + 2) * TB < steps + TB - 1 and (blk + 2) * TB < T:
                        nc.sync.dma_start(
                            out=xwr[:, blk % 2],
                            in_=d_xw[:, :, (blk + 2) * TB:(blk + 3) * TB, :])
                if t == steps - 1 and steps % TB != 0:
                    blk = t // TB
                    nc.sync.dma_start(
                        out=d_y[:, blk * TB:blk * TB + (steps % TB), :, :],
                        in_=yring[:, (blk % 2) * TB:
                                  (blk % 2) * TB + (steps % TB)])

        _st.close()

    nc.compile()
    _cache[key] = nc
    return nc


def _prepare(x, A, Wx, Wh, Wattn, b):
    x = np.asarray(x, dtype=np.float32)
    A = np.ascontiguousarray(np.asarray(A, dtype=np.float32))
    Wxb = np.ascontiguousarray(
        np.asarray(Wx, dtype=np.float32).astype(np.float16))
    Whb = np.ascontiguousarray(
        np.asarray(Wh, dtype=np.float32).astype(np.float16))
    Wab = np.ascontiguousarray(
        np.asarray(Wattn, dtype=np.float32).astype(np.float16))
    bT = np.ascontiguousarray(
        np.asarray(b, dtype=np.float32).reshape(JC, 128).T)

    id128 = np.eye(128, dtype=np.float16)
    # selc[m, h, q] = 1 iff m == q % 16   (q local (n,m) index in half h)
    selc = np.zeros((16, 2, 128), dtype=np.float16)
    for hh in range(2):
        for q in range(128):
            selc[q % 16, hh, q] = 1.0
    selc = selc.reshape(16, 256)
    # maskD[q, h, n'] = 1 iff n' == 8h + q // 16
    maskD = np.zeros((128, 2, NL), dtype=np.float16)
    for q in range(128):
        for hh in range(2):
            maskD[q, hh, 8 * hh + q // 16] = 1.0
    maskD = maskD.reshape(128, 2 * NL)
    maskdiag = np.eye(16, dtype=np.float32)

    in_maps = []
    for k in range(NCORES):
        xs = x[k * NL:(k + 1) * NL]                      # [16, 64, 1024]
        xT = np.ascontiguousarray(
            xs.transpose(1, 0, 2).reshape(T * NL, D).T.astype(np.float16))
        Ak = np.ascontiguousarray(A[k * NL:(k + 1) * NL].reshape(NL, H, M))
        in_maps.append({
            "xT": xT, "A": Ak, "Wx": Wxb, "Wh": Whb, "Wa": Wab,
            "bT": bT, "id128": id128, "selc": selc, "maskD": maskD,
            "maskdiag": maskdiag,
        })

    _cache["in_maps"] = in_maps
    return in_maps


def kernel(x, A, Wx, Wh, Wattn, b):
    nc = _build()
    in_maps = _prepare(x, A, Wx, Wh, Wattn, b)
    res = run_bass_kernel_spmd(nc, in_maps, core_ids=list(range(NCORES)))
    outs = []
    for k in range(NCORES):
        yk = res.results[k]["y"]                          # [128, T, KC, NL]
        outs.append(yk.transpose(3, 1, 2, 0).reshape(NL, T, H))
    return np.concatenate(outs, axis=0).astype(np.float32)



# revision 25
# speedup vs baseline: 1.0003x; 1.0003x over previous
"""Attention-LSTM (CaptioningRNN) Trainium2 kernel, v2.

Data-parallel over N=128 across 8 cores (16 samples each), with every
per-step tensor kept TRANSPOSED (feature dim on the 128 partitions, the
16 local samples on the free dim) so each matmul's output free size is
16 and the gate GEMMs run weight-stationary:

  act^T[j, n] = sum_k Wh[k, j] h^T[k, n]          (256 matmuls, ap16)
              + sum_(n',m) AW2[(n',m), j] wdT[(n',m), n]   (64 matmuls)
              + xW^T[j, (t, n)]                   (1 identity matmul)

where AW2[(n,m), j] = sum_h A[n, h, m] Wattn[h, j] is precomputed once
(phase 1), which turns the whole attention-apply + attn@Wattn step into
a real 256-deep contraction: wdT[(n',m), n] = softmax_w[n, m] * (n'==n)
is built by a tiny scatter-matmul from the softmax weights.

Phase 1 also computes xW^T = (x @ Wx + b)^T into a DRAM scratch in a
layout that lets each step load its slice with large descriptors.

h/c state lives as [128 (h mod 128), 8 (h chunk), 16 (n)]; h^T is
written directly into the y-store ring buffer that both the next step's
matmuls and the (batched) y DMA read.
"""

import sys
from contextlib import ExitStack

sys.path.insert(0, "/opt/trn_rl_repo")

import numpy as np

import concourse.bass as bass  # noqa: F401
import concourse.mybir as mybir
import concourse.tile as tile
from concourse import bacc
from concourse.bass_utils import run_bass_kernel_spmd

F32 = mybir.dt.float32
BF16 = mybir.dt.float16  # IEEE fp16: same PE rate as bf16, more mantissa

N, T, D, H = 128, 64, 1024, 1024
K4 = 4 * H            # 4096
NCORES = 8
NL = N // NCORES      # 16 samples per core
M = 16                # spatial positions (4x4)
KC = H // 128         # 8 contraction chunks
JC = K4 // 128        # 32 output chunks
SCALE = 1.0 / float(np.sqrt(H))
TB = 4                # steps per xw-load / y-store block
N_WARM = 16
WARM_EN = False
GATE_MODE = 'four'
G_FIRST = True
EXP_ACCUM = True
GATE_COPY = False

Alu = mybir.AluOpType
Act = mybir.ActivationFunctionType

_cache = {}


def _build(steps=T):
    key = ("nc", steps)
    if key in _cache:
        return _cache[key]

    nc = bacc.Bacc("TRN2", target_bir_lowering=False)

    # ---- kernel I/O ----------------------------------------------------
    d_xT = nc.dram_tensor("xT", [D, T * NL], BF16, kind="ExternalInput")
    d_A = nc.dram_tensor("A", [NL, H, M], F32, kind="ExternalInput")
    d_Wx = nc.dram_tensor("Wx", [D, K4], BF16, kind="ExternalInput")
    d_Wh = nc.dram_tensor("Wh", [H, K4], BF16, kind="ExternalInput")
    d_Wa = nc.dram_tensor("Wa", [H, K4], BF16, kind="ExternalInput")
    d_bT = nc.dram_tensor("bT", [128, JC], F32, kind="ExternalInput")
    d_id = nc.dram_tensor("id128", [128, 128], BF16, kind="ExternalInput")
    d_qsel = nc.dram_tensor("qsel", [128, 8], BF16, kind="ExternalInput")
    d_qselT = nc.dram_tensor("qselT", [8, 128], BF16, kind="ExternalInput")
    d_mD = nc.dram_tensor("maskD", [128, 2 * 16], BF16, kind="ExternalInput")
    # outputs / scratch
    d_y = nc.dram_tensor("y", [128, T, KC, NL], BF16, kind="ExternalOutput")
    d_xw = nc.dram_tensor("xw", [128, JC, T, NL], BF16)

    with tile.TileContext(nc) as tc:
      with (
          tc.tile_pool(name="wts", bufs=1) as wts,    # persistent weights
          tc.tile_pool(name="stt", bufs=1) as stt,    # persistent state
      ):
        # ---------------- persistent SBUF tensors -----------------------
        at4 = wts.tile([128, KC, NL, M], BF16, tag="at4")
        aw2 = wts.tile([128, 2, JC, 128], BF16, tag="aw2")
        id128 = wts.tile([128, 128], BF16, tag="id128")
        qsel = wts.tile([128, 8], BF16, tag="qsel")
        qselT = wts.tile([8, 128], BF16, tag="qselT")
        maskD = wts.tile([128, 2, NL], BF16, tag="maskD")
        bT = wts.tile([128, JC], F32, tag="bT")

        c_sb = stt.tile([128, KC, NL], F32, tag="c")
        hT0 = stt.tile([128, KC, NL], BF16, tag="hT0")
        # y ring: 2 buffers x TB steps; h^T of step t lives in
        # yring[t % 2TB]; matmuls of step t+1 read it, y DMA stores it.
        yring = stt.tile([128, 2 * TB, KC, NL], BF16, tag="yring")
        g_if = stt.tile([128, 2, KC, NL], F32, tag="g_if")
        g_o = stt.tile([128, KC, NL], F32, tag="g_o")
        g_g = stt.tile([128, KC, NL], F32, tag="g_g")
        th = stt.tile([128, KC, NL], F32, tag="th")
        # softmax state, all at q=(n,m) partitions (2 h-halves)
        junkq = stt.tile([128, 2, NL], F32, tag="junkq")
        scq = stt.tile([128, 2], F32, tag="scq")
        rrq = stt.tile([128, 2], F32, tag="rrq")
        uq = stt.tile([128, 2], BF16, tag="uq")
        rsum2 = stt.tile([8, 2], BF16, tag="rsum2")
        wq = stt.tile([128, 2], F32, tag="wq")
        wdT = stt.tile([128, 2, NL], BF16, tag="wdT")
        xwr = stt.tile([128, 2, JC, TB, NL], BF16, tag="xwr")


        # ====== Phase 1: xW^T GEMM + AW GEMM, DMA-pipelined =============
        # DMA issue order = consumption order: xt, Wx j-slices (xW GEMM
        # starts after ~2 slices), A chunks, Wa j-slices (AW GEMM
        # pipelines), then Wh into Wa's freed space (not needed until
        # the first recurrent step, so its load hides under the GEMMs).
        _st = ExitStack()
        whp = _st.enter_context(tc.tile_pool(name="whp", bufs=1))
        wh = whp.tile([128, KC, K4], BF16, tag="wh")
        p1bw = _st.enter_context(tc.tile_pool(name="p1bw", bufs=1))
        p1wx = _st.enter_context(tc.tile_pool(name="p1wx", bufs=5))
        p1bs = _st.enter_context(tc.tile_pool(name="p1bs", bufs=14))
        p1a = _st.enter_context(tc.tile_pool(name="p1a", bufs=8))
        if True:
            xt = p1bw.tile([128, KC, T * NL], BF16, tag="xt")
            d_xT_r = d_xT.rearrange("(kc p) r -> p kc r", p=128)
            d_Wx_r = d_Wx.rearrange("(kc p) f -> p kc f", p=128)
            wxs = []

            def load_wx(jb):
                wxj = p1wx.tile([128, KC, 512], BF16, tag="wx")
                nc.sync.dma_start(out=wxj[:],
                                  in_=d_Wx_r[:, :, jb * 512:(jb + 1) * 512])
                wxs.append(wxj)

            nc.sync.dma_start(out=xt[:, :, 0:512], in_=d_xT_r[:, :, 0:512])
            load_wx(0)
            nc.sync.dma_start(out=xt[:, :, 512:1024],
                              in_=d_xT_r[:, :, 512:1024])
            for jb in range(1, 8):
                load_wx(jb)
            a_sts = []
            for kc in range(KC):
                a_st = p1a.tile([128, NL, M], F32, tag="a_st")
                nc.sync.dma_start(
                    out=a_st[:],
                    in_=d_A.rearrange("n (kc p) m -> kc p n m", p=128)[kc])
                a_sts.append(a_st)
            # small-const loads
            nc.sync.dma_start(out=id128[:], in_=d_id[:])
            nc.sync.dma_start(out=qsel[:], in_=d_qsel[:])
            nc.sync.dma_start(out=qselT[:], in_=d_qselT[:])
            nc.sync.dma_start(
                out=maskD[:], in_=d_mD.rearrange("p (h n) -> p h n", h=2))
            nc.sync.dma_start(out=bT[:], in_=d_bT[:])

            with (
                tc.tile_pool(name="p1w", bufs=3) as p1w,
                tc.tile_pool(name="p1bp", bufs=6, space="PSUM") as p1bp,
            ):
                # at4 (bf16) and c0 = mean_m A from the staged chunks
                for kc in range(KC):
                    nc.vector.tensor_copy(at4[:, kc], a_sts[kc][:])
                    nc.vector.tensor_reduce(
                        c_sb[:, kc, :], a_sts[kc][:],
                        axis=mybir.AxisListType.X, op=Alu.add)
                nc.vector.tensor_scalar_mul(hT0[:], c_sb[:], 1.0 / M)
                nc.scalar.mul(c_sb[:], c_sb[:], 1.0 / M)

                # xW^T GEMM first (chases the Wx slice DMAs), then the
                # AW GEMM (chases the Wa slices) — strictly in PE order.
                for jb in range(8):
                    for jc in range(4 * jb, 4 * jb + 4):
                        for tb in range(2):
                            pxw = p1bp.tile([128, 512], F32, tag="pxw")
                            for kc in range(KC):
                                nc.tensor.matmul(
                                    pxw[:],
                                    wxs[jb][:, kc,
                                            (jc - 4 * jb) * 128:
                                            (jc - 4 * jb + 1) * 128],
                                    xt[:, kc, tb * 512:(tb + 1) * 512],
                                    start=(kc == 0), stop=(kc == KC - 1))
                            ob = p1bs.tile([128, 512], BF16, tag="p1out")
                            nc.vector.tensor_scalar_add(
                                ob[:], pxw[:], bT[:, jc:jc + 1])
                            nc.sync.dma_start(
                                out=d_xw[:, jc, tb * 32:(tb + 1) * 32, :]
                                .rearrange("p t n -> p (t n)"),
                                in_=ob[:])
                d_Wa_r = d_Wa.rearrange("(kc p) f -> p kc f", p=128)
                was = []
                for jb in range(8):
                    waj = p1w.tile([128, KC, 512], BF16, tag="wa")
                    nc.sync.dma_start(
                        out=waj[:],
                        in_=d_Wa_r[:, :, jb * 512:(jb + 1) * 512])
                    was.append(waj)
                # Wh load (sliced so it interleaves with the ring
                # reloads on the serial DMA device) overlaps phase 1
                d_Wh_r = d_Wh.rearrange("(kc p) f -> p kc f", p=128)
                for jb in range(8):
                    nc.sync.dma_start(
                        out=wh[:, :, jb * 512:(jb + 1) * 512],
                        in_=d_Wh_r[:, :, jb * 512:(jb + 1) * 512])

                for jb in range(8):
                    for h in range(2):
                        paw = p1bp.tile([128, 512], F32, tag="pxw")
                        for kc in range(KC):
                            nc.tensor.matmul(
                                paw[:],
                                at4[:, kc, 8 * h:8 * h + 8, :].rearrange(
                                    "p n m -> p (n m)"),
                                was[jb][:, kc, :],
                                start=(kc == 0), stop=(kc == KC - 1))
                        nc.vector.tensor_copy(
                            aw2[:, h, 4 * jb:4 * jb + 4, :].rearrange(
                                "p jc q -> p (jc q)"),
                            paw[:])

        # ================== Phase 2: recurrent steps ====================
        actp = _st.enter_context(tc.tile_pool(name="actp", bufs=3, space="PSUM"))
        scp = _st.enter_context(tc.tile_pool(name="scp", bufs=1, space="PSUM"))
        smls = _st.enter_context(tc.tile_pool(name="smls", bufs=2))
        if True:
            def warm_on(lhsT, rhs, k):
                if not WARM_EN:
                    return
                """Low-priority PE filler matmuls that become ready only
                once `lhsT` is written: they bridge PE idle gaps in the
                recurrent chain so the p-state ramp stays hot, yielding
                to any ready real matmul (which carries higher priority).
                """
                p = int(np.prod(lhsT.shape[1:]))
                f = int(np.prod(rhs.shape[1:]))
                for _ in range(k):
                    nc.tensor.matmul(
                        pdum[0:p, 0:f], lhsT, rhs,
                        start=True, stop=True)

            # prefetch xw for t-blocks 0 and 1
            for b in range(2):
                nc.sync.dma_start(
                    out=xwr[:, b], in_=d_xw[:, :, b * TB:(b + 1) * TB, :])

            # per-gate act psum tiles: consecutive ACT reads of one
            # psum tile serialize at +219ns each; separate tiles pipeline
            pact = [None, None, None]

            def alloc_bank():
                # all four gates in one 2KB psum bank: [i, f, o, g]
                return actp.tile([128, 4, KC, NL], F32, tag="pact",
                                 name="pact")

            def cur_slice(bank, jc):
                return bank[:, jc // 8, jc % 8, :]

            def xw_add(bank, tt):
                # exactly ONE start=True per psum bank: start marks the
                # whole 2KB zero region pending-zero, so a second start
                # would wipe the first half's data
                for half in range(2):
                    nc.tensor.matmul(
                        bank[:, 2 * half:2 * half + 2],
                        id128[:],
                        xwr[:, tt // TB % 2, 16 * half:16 * half + 16,
                            tt % TB, :],
                        start=(half == 0), stop=False,
                        skip_group_check=True)

            for tt in range(min(2, steps)):
                bank = alloc_bank()
                pact[tt % 3] = bank
                xw_add(bank, tt)

            if G_FIRST:
                JC_ORDER = list(range(24, 32)) + list(range(0, 24))
            else:
                JC_ORDER = list(range(0, 16)) + list(range(24, 32)) + \
                    list(range(16, 24))

            for t in range(steps):
                cur = pact[t % 3]
                hT = hT0 if t == 0 else yring[:, (t - 1) % (2 * TB)]

                # -- scores at q=(n,m) partitions: psc2[q, h, n'] =
                #    sum_k A[k, q] h[k, n'].  Output free dim is only n'
                #    (16), so the 16 matmuls cost ~7ns each instead of a
                #    [16, 256]-wide stream.
                psc2 = scp.tile([128, 2, NL], F32, tag="psc2")
                for kc in range(KC):
                    for h in range(2):
                        nc.tensor.matmul(
                            psc2[:, h, :],
                            at4[:, kc, 8 * h:8 * h + 8, :].rearrange(
                                "p n m -> p (n m)"),
                            hT[:, kc, :],
                            start=(kc == 0 and h == 0),
                            stop=(kc == KC - 1),
                            skip_group_check=True)

                # -- Wh gate matmuls (independent of attention chain)
                for jc in range(JC):
                    for kc in range(KC):
                        nc.tensor.matmul(
                            cur_slice(cur, jc),
                            wh[:, kc, jc * 128:(jc + 1) * 128],
                            hT[:, kc, :], start=False, stop=False,
                            skip_group_check=True)

                # -- diag-extract scores per partition q: mask-multiply,
                #    then reduce the n' axis (innermost), keeping h.
                #    (TensorTensorReduce would fuse these but wedges the
                #    device under this runtime.)
                nc.vector.tensor_tensor(
                    junkq[:], psc2[:], maskD[:], op=Alu.mult)
                nc.vector.tensor_reduce(
                    scq[:], junkq[:], axis=mybir.AxisListType.X, op=Alu.add)
                # exp(x) = 1/sigmoid(-x) - 1: keeps every activation in
                # the 'sigmoid_and_others' HW table (sigmoid+tanh), so no
                # 1283ns act-table reload is needed anywhere in the loop.
                # |score*scale| <= ~3, so sigmoid never saturates and the
                # r-1 cancellation only affects negligibly small weights.
                nc.scalar.activation(
                    rrq[:], scq[:], Act.Sigmoid, scale=-SCALE)
                nc.vector.reciprocal(scq[:], rrq[:])
                nc.vector.tensor_scalar(
                    uq[:], scq[:], -1.0, 0.0, op0=Alu.add, op1=Alu.add)

                # -- softmax sum over m per sample n via two tiny matmuls
                #    (cross-partition add), reciprocal, then broadcast the
                #    1/sum back to q partitions with two more matmuls.
                pss = scp.tile([8, 2], F32, tag="pss")
                for h in range(2):
                    nc.tensor.matmul(
                        pss[:, h:h + 1], qsel[:], uq[:, h:h + 1],
                        start=(h == 0), stop=(h == 1),
                        skip_group_check=True)
                with nc.allow_low_precision(
                        reason="1/softmax-sum feeds a bf16 matmul anyway"):
                    nc.vector.reciprocal(rsum2[:], pss[:])
                rsb = scp.tile([128, 2], F32, tag="rsb")
                for h in range(2):
                    nc.tensor.matmul(
                        rsb[:, h:h + 1], qselT[:], rsum2[:, h:h + 1],
                        start=(h == 0), stop=(h == 1),
                        skip_group_check=True)
                # xw identity-add for step t+2 fills the PE stall while
                # the tail of the softmax chain runs on DVE
                if t + 2 < steps:
                    nxt = alloc_bank()
                    pact[(t + 2) % 3] = nxt
                    xw_add(nxt, t + 2)
                nc.vector.tensor_tensor(
                    wq[:], uq[:], rsb[:], op=Alu.mult)
                nc.vector.tensor_tensor(
                    wdT[:], maskD[:],
                    wq[:].unsqueeze(2).broadcast_to([128, 2, NL]),
                    op=Alu.mult)

                # -- attention gate matmuls close each jc's psum group
                # (o-gate jc 16..23 last so its ACT is off the c-path)
                for jc in JC_ORDER:
                    for h in range(2):
                        nc.tensor.matmul(
                            cur_slice(cur, jc),
                            aw2[:, h, jc, :], wdT[:, h, :],
                            start=False, stop=(h == 1),
                            skip_group_check=True)

                # -- gate activations: tanh(g) first (its attn matmuls
                # close first), then ONE sigmoid over the contiguous
                # i/f block, then o last (off the c-path).
                nc.scalar.activation(g_g[:], cur[:, 3], Act.Tanh)
                nc.scalar.activation(g_if[:], cur[:, 0:2], Act.Sigmoid)
                nc.scalar.activation(g_o[:], cur[:, 2], Act.Sigmoid)

                # -- c = f*c + i*g ; h = o*tanh(c)
                fc = smls.tile([128, KC, NL], F32, tag="fc")
                nc.vector.tensor_tensor(
                    fc[:], g_if[:, 1], c_sb[:], op=Alu.mult)
                ig = smls.tile([128, KC, NL], F32, tag="ig")
                nc.vector.tensor_tensor(
                    ig[:], g_if[:, 0], g_g[:], op=Alu.mult)
                nc.vector.tensor_tensor(c_sb[:], fc[:], ig[:], op=Alu.add)
                nc.scalar.activation(th[:], c_sb[:], Act.Tanh)
                hout = yring[:, t % (2 * TB)]
                for q in range(4):
                    nc.vector.tensor_tensor(
                        hout[:, 2 * q:2 * q + 2], g_o[:, 2 * q:2 * q + 2],
                        th[:, 2 * q:2 * q + 2], op=Alu.mult)

                # -- batched y store + xw prefetch every TB steps
                if t % TB == TB - 1:
                    blk = t // TB
                    nc.sync.dma_start(
                        out=d_y[:, blk * TB:(blk + 1) * TB, :, :],
                        in_=yring[:, (blk % 2) * TB:(blk % 2) * TB + TB])
                    if (blk + 2) * TB < steps + TB - 1 and (blk + 2) * TB < T:
                        nc.sync.dma_start(
                            out=xwr[:, blk % 2],
                            in_=d_xw[:, :, (blk + 2) * TB:(blk + 3) * TB, :])
                if t == steps - 1 and steps % TB != 0:
                    blk = t // TB
                    nc.sync.dma_start(
                        out=d_y[:, blk * TB:blk * TB + (steps % TB), :, :],
                        in_=yring[:, (blk % 2) * TB:
                                  (blk % 2) * TB + (steps % TB)])

        _st.close()

    nc.compile()
    _cache[key] = nc
    return nc


def _prepare(x, A, Wx, Wh, Wattn, b):
    x = np.asarray(x, dtype=np.float32)
    A = np.ascontiguousarray(np.asarray(A, dtype=np.float32))
    Wxb = np.ascontiguousarray(
        np.asarray(Wx, dtype=np.float32).astype(np.float16))
    Whb = np.ascontiguousarray(
        np.asarray(Wh, dtype=np.float32).astype(np.float16))
    Wab = np.ascontiguousarray(
        np.asarray(Wattn, dtype=np.float32).astype(np.float16))
    bT = np.ascontiguousarray(
        np.asarray(b, dtype=np.float32).reshape(JC, 128).T)

    id128 = np.eye(128, dtype=np.float16)
    # qsel[q, g] = 1 iff q // 16 == g  (sums the 16 m-positions of local
    # sample-group g); qselT is its transpose (broadcast back to q)
    qsel = np.zeros((128, 8), dtype=np.float16)
    for q in range(128):
        qsel[q, q // 16] = 1.0
    qselT = np.ascontiguousarray(qsel.T)
    # maskD[q, h, n'] = 1 iff n' == 8h + q // 16
    maskD = np.zeros((128, 2, NL), dtype=np.float16)
    for q in range(128):
        for hh in range(2):
            maskD[q, hh, 8 * hh + q // 16] = 1.0
    maskD = maskD.reshape(128, 2 * NL)

    in_maps = []
    for k in range(NCORES):
        xs = x[k * NL:(k + 1) * NL]                      # [16, 64, 1024]
        xT = np.ascontiguousarray(
            xs.transpose(1, 0, 2).reshape(T * NL, D).T.astype(np.float16))
        Ak = np.ascontiguousarray(A[k * NL:(k + 1) * NL].reshape(NL, H, M))
        in_maps.append({
            "xT": xT, "A": Ak, "Wx": Wxb, "Wh": Whb, "Wa": Wab,
            "bT": bT, "id128": id128, "qsel": qsel, "qselT": qselT,
            "maskD": maskD,
        })

    _cache["in_maps"] = in_maps
    return in_maps


def kernel(x, A, Wx, Wh, Wattn, b):
    nc = _build()
    in_maps = _prepare(x, A, Wx, Wh, Wattn, b)
    res = run_bass_kernel_spmd(nc, in_maps, core_ids=list(range(NCORES)))
    outs = []
    for k in range(NCORES):
        yk = res.results[k]["y"]                          # [128, T, KC, NL]
        outs.append(yk.transpose(3, 1, 2, 0).reshape(NL, T, H))
    return np.concatenate(outs, axis=0).astype(np.float32)



# revision 29
# speedup vs baseline: 1.0523x; 1.0520x over previous
"""Attention-LSTM (CaptioningRNN) Trainium2 kernel, v2.

Data-parallel over N=128 across 8 cores (16 samples each), with every
per-step tensor kept TRANSPOSED (feature dim on the 128 partitions, the
16 local samples on the free dim) so each matmul's output free size is
16 and the gate GEMMs run weight-stationary:

  act^T[j, n] = sum_k Wh[k, j] h^T[k, n]          (256 matmuls, ap16)
              + sum_(n',m) AW2[(n',m), j] wdT[(n',m), n]   (64 matmuls)
              + xW^T[j, (t, n)]                   (1 identity matmul)

where AW2[(n,m), j] = sum_h A[n, h, m] Wattn[h, j] is precomputed once
(phase 1), which turns the whole attention-apply + attn@Wattn step into
a real 256-deep contraction: wdT[(n',m), n] = softmax_w[n, m] * (n'==n)
is built by a tiny scatter-matmul from the softmax weights.

Phase 1 also computes xW^T = (x @ Wx + b)^T into a DRAM scratch in a
layout that lets each step load its slice with large descriptors.

h/c state lives as [128 (h mod 128), 8 (h chunk), 16 (n)]; h^T is
written directly into the y-store ring buffer that both the next step's
matmuls and the (batched) y DMA read.
"""

import sys
from contextlib import ExitStack

sys.path.insert(0, "/opt/trn_rl_repo")

import numpy as np

import concourse.bass as bass  # noqa: F401
import concourse.mybir as mybir
import concourse.tile as tile
from concourse import bacc
from concourse.bass_utils import run_bass_kernel_spmd

F32 = mybir.dt.float32
BF16 = mybir.dt.float16  # IEEE fp16: same PE rate as bf16, more mantissa

N, T, D, H = 128, 64, 1024, 1024
K4 = 4 * H            # 4096
NCORES = 8
NL = N // NCORES      # 16 samples per core
M = 16                # spatial positions (4x4)
KC = H // 128         # 8 contraction chunks
JC = K4 // 128        # 32 output chunks
SCALE = 1.0 / float(np.sqrt(H))
TB = 4                # steps per xw-load / y-store block
N_WARM = 16
WARM_EN = False
GATE_MODE = 'four'
G_FIRST = True
EXP_ACCUM = True
GATE_COPY = False

Alu = mybir.AluOpType
Act = mybir.ActivationFunctionType

_cache = {}


def _build(steps=T):
    key = ("nc", steps)
    if key in _cache:
        return _cache[key]

    nc = bacc.Bacc("TRN2", target_bir_lowering=False)

    # ---- kernel I/O ----------------------------------------------------
    d_xT = nc.dram_tensor("xT", [D, T * NL], BF16, kind="ExternalInput")
    d_A = nc.dram_tensor("A", [NL, H, M], F32, kind="ExternalInput")
    d_Wx = nc.dram_tensor("Wx", [D, K4], BF16, kind="ExternalInput")
    d_Wh = nc.dram_tensor("Wh", [H, K4], BF16, kind="ExternalInput")
    d_Wa = nc.dram_tensor("Wa", [H, K4], BF16, kind="ExternalInput")
    d_bT = nc.dram_tensor("bT", [128, JC], F32, kind="ExternalInput")
    d_id = nc.dram_tensor("id128", [128, 128], BF16, kind="ExternalInput")
    d_qsel = nc.dram_tensor("qsel", [128, 8], BF16, kind="ExternalInput")
    d_qselT = nc.dram_tensor("qselT", [8, 128], BF16, kind="ExternalInput")
    d_mD = nc.dram_tensor("maskD", [128, 2 * 16], BF16, kind="ExternalInput")
    # outputs / scratch
    d_y = nc.dram_tensor("y", [128, T, KC, NL], BF16, kind="ExternalOutput")
    d_xw = nc.dram_tensor("xw", [128, JC, T, NL], BF16)

    with tile.TileContext(nc) as tc:
      with (
          tc.tile_pool(name="wts", bufs=1) as wts,    # persistent weights
          tc.tile_pool(name="stt", bufs=1) as stt,    # persistent state
      ):
        # ---------------- persistent SBUF tensors -----------------------
        at4 = wts.tile([128, KC, NL, M], BF16, tag="at4")
        aw2 = wts.tile([128, 2, JC, 128], BF16, tag="aw2")
        id128 = wts.tile([128, 128], BF16, tag="id128")
        qsel = wts.tile([128, 8], BF16, tag="qsel")
        qselT = wts.tile([8, 128], BF16, tag="qselT")
        maskD = wts.tile([128, 2, NL], BF16, tag="maskD")
        bT = wts.tile([128, JC], F32, tag="bT")

        c_sb = stt.tile([128, KC, NL], F32, tag="c")
        hT0 = stt.tile([128, KC, NL], BF16, tag="hT0")
        # y ring: 2 buffers x TB steps; h^T of step t lives in
        # yring[t % 2TB]; matmuls of step t+1 read it, y DMA stores it.
        yring = stt.tile([128, 2 * TB, KC, NL], BF16, tag="yring")
        g_if = stt.tile([128, 2, KC, NL], F32, tag="g_if")
        g_o = stt.tile([128, KC, NL], F32, tag="g_o")
        g_g = stt.tile([128, KC, NL], F32, tag="g_g")
        th = stt.tile([128, KC, NL], F32, tag="th")
        # softmax state, all at q=(n,m) partitions (2 h-halves)
        junkq = stt.tile([128, 2, NL], F32, tag="junkq")
        scq = stt.tile([128, 2], F32, tag="scq")
        rrq = stt.tile([128, 2], F32, tag="rrq")
        uq = stt.tile([128, 2], BF16, tag="uq")
        rsum2 = stt.tile([8, 2], BF16, tag="rsum2")
        wdTun = stt.tile([128, 2, NL], BF16, tag="wdTun")
        wdT = stt.tile([128, 2, NL], BF16, tag="wdT")
        xwr = stt.tile([128, 2, JC, TB, NL], BF16, tag="xwr")


        # ====== Phase 1: xW^T GEMM + AW GEMM, DMA-pipelined =============
        # DMA issue order = consumption order: xt, Wx j-slices (xW GEMM
        # starts after ~2 slices), A chunks, Wa j-slices (AW GEMM
        # pipelines), then Wh into Wa's freed space (not needed until
        # the first recurrent step, so its load hides under the GEMMs).
        _st = ExitStack()
        whp = _st.enter_context(tc.tile_pool(name="whp", bufs=1))
        wh = whp.tile([128, KC, K4], BF16, tag="wh")
        p1bw = _st.enter_context(tc.tile_pool(name="p1bw", bufs=1))
        p1wx = _st.enter_context(tc.tile_pool(name="p1wx", bufs=5))
        p1bs = _st.enter_context(tc.tile_pool(name="p1bs", bufs=14))
        p1a = _st.enter_context(tc.tile_pool(name="p1a", bufs=8))
        if True:
            xt = p1bw.tile([128, KC, T * NL], BF16, tag="xt")
            d_xT_r = d_xT.rearrange("(kc p) r -> p kc r", p=128)
            d_Wx_r = d_Wx.rearrange("(kc p) f -> p kc f", p=128)
            wxs = []

            def load_wx(jb):
                wxj = p1wx.tile([128, KC, 512], BF16, tag="wx")
                nc.sync.dma_start(out=wxj[:],
                                  in_=d_Wx_r[:, :, jb * 512:(jb + 1) * 512])
                wxs.append(wxj)

            nc.sync.dma_start(out=xt[:, :, 0:512], in_=d_xT_r[:, :, 0:512])
            load_wx(0)
            nc.sync.dma_start(out=xt[:, :, 512:1024],
                              in_=d_xT_r[:, :, 512:1024])
            for jb in range(1, 8):
                load_wx(jb)
            a_sts = []
            for kc in range(KC):
                a_st = p1a.tile([128, NL, M], F32, tag="a_st")
                nc.sync.dma_start(
                    out=a_st[:],
                    in_=d_A.rearrange("n (kc p) m -> kc p n m", p=128)[kc])
                a_sts.append(a_st)
            # small-const loads
            nc.sync.dma_start(out=id128[:], in_=d_id[:])
            nc.sync.dma_start(out=qsel[:], in_=d_qsel[:])
            nc.sync.dma_start(out=qselT[:], in_=d_qselT[:])
            nc.sync.dma_start(
                out=maskD[:], in_=d_mD.rearrange("p (h n) -> p h n", h=2))
            nc.sync.dma_start(out=bT[:], in_=d_bT[:])

            with (
                tc.tile_pool(name="p1w", bufs=3) as p1w,
                tc.tile_pool(name="p1bp", bufs=6, space="PSUM") as p1bp,
            ):
                # at4 (bf16) and c0 = mean_m A from the staged chunks
                for kc in range(KC):
                    nc.vector.tensor_copy(at4[:, kc], a_sts[kc][:])
                    nc.vector.tensor_reduce(
                        c_sb[:, kc, :], a_sts[kc][:],
                        axis=mybir.AxisListType.X, op=Alu.add)
                nc.vector.tensor_scalar_mul(hT0[:], c_sb[:], 1.0 / M)
                nc.scalar.mul(c_sb[:], c_sb[:], 1.0 / M)

                # xW^T GEMM first (chases the Wx slice DMAs), then the
                # AW GEMM (chases the Wa slices) — strictly in PE order.
                for jb in range(8):
                    for jc in range(4 * jb, 4 * jb + 4):
                        for tb in range(2):
                            pxw = p1bp.tile([128, 512], F32, tag="pxw")
                            for kc in range(KC):
                                nc.tensor.matmul(
                                    pxw[:],
                                    wxs[jb][:, kc,
                                            (jc - 4 * jb) * 128:
                                            (jc - 4 * jb + 1) * 128],
                                    xt[:, kc, tb * 512:(tb + 1) * 512],
                                    start=(kc == 0), stop=(kc == KC - 1))
                            ob = p1bs.tile([128, 512], BF16, tag="p1out")
                            nc.vector.tensor_scalar_add(
                                ob[:], pxw[:], bT[:, jc:jc + 1])
                            nc.sync.dma_start(
                                out=d_xw[:, jc, tb * 32:(tb + 1) * 32, :]
                                .rearrange("p t n -> p (t n)"),
                                in_=ob[:])
                d_Wa_r = d_Wa.rearrange("(kc p) f -> p kc f", p=128)
                was = []
                for jb in range(8):
                    waj = p1w.tile([128, KC, 512], BF16, tag="wa")
                    nc.sync.dma_start(
                        out=waj[:],
                        in_=d_Wa_r[:, :, jb * 512:(jb + 1) * 512])
                    was.append(waj)
                # Wh load (sliced so it interleaves with the ring
                # reloads on the serial DMA device) overlaps phase 1
                d_Wh_r = d_Wh.rearrange("(kc p) f -> p kc f", p=128)
                for jb in range(8):
                    nc.sync.dma_start(
                        out=wh[:, :, jb * 512:(jb + 1) * 512],
                        in_=d_Wh_r[:, :, jb * 512:(jb + 1) * 512])

                for jb in range(8):
                    for h in range(2):
                        paw = p1bp.tile([128, 512], F32, tag="pxw")
                        for kc in range(KC):
                            nc.tensor.matmul(
                                paw[:],
                                at4[:, kc, 8 * h:8 * h + 8, :].rearrange(
                                    "p n m -> p (n m)"),
                                was[jb][:, kc, :],
                                start=(kc == 0), stop=(kc == KC - 1))
                        nc.vector.tensor_copy(
                            aw2[:, h, 4 * jb:4 * jb + 4, :].rearrange(
                                "p jc q -> p (jc q)"),
                            paw[:])

        # ================== Phase 2: recurrent steps ====================
        actp = _st.enter_context(tc.tile_pool(name="actp", bufs=3, space="PSUM"))
        scp = _st.enter_context(tc.tile_pool(name="scp", bufs=1, space="PSUM"))
        smls = _st.enter_context(tc.tile_pool(name="smls", bufs=2))
        if True:
            def warm_on(lhsT, rhs, k):
                if not WARM_EN:
                    return
                """Low-priority PE filler matmuls that become ready only
                once `lhsT` is written: they bridge PE idle gaps in the
                recurrent chain so the p-state ramp stays hot, yielding
                to any ready real matmul (which carries higher priority).
                """
                p = int(np.prod(lhsT.shape[1:]))
                f = int(np.prod(rhs.shape[1:]))
                for _ in range(k):
                    nc.tensor.matmul(
                        pdum[0:p, 0:f], lhsT, rhs,
                        start=True, stop=True)

            # prefetch xw for t-blocks 0 and 1
            for b in range(2):
                nc.sync.dma_start(
                    out=xwr[:, b], in_=d_xw[:, :, b * TB:(b + 1) * TB, :])

            # per-gate act psum tiles: consecutive ACT reads of one
            # psum tile serialize at +219ns each; separate tiles pipeline
            pact = [None, None, None]

            def alloc_bank():
                # all four gates in one 2KB psum bank: [i, f, o, g]
                return actp.tile([128, 4, KC, NL], F32, tag="pact",
                                 name="pact")

            def cur_slice(bank, jc):
                return bank[:, jc // 8, jc % 8, :]

            def xw_add(bank, tt):
                # exactly ONE start=True per psum bank: start marks the
                # whole 2KB zero region pending-zero, so a second start
                # would wipe the first half's data
                for half in range(2):
                    nc.tensor.matmul(
                        bank[:, 2 * half:2 * half + 2],
                        id128[:],
                        xwr[:, tt // TB % 2, 16 * half:16 * half + 16,
                            tt % TB, :],
                        start=(half == 0), stop=False,
                        skip_group_check=True)

            for tt in range(min(2, steps)):
                bank = alloc_bank()
                pact[tt % 3] = bank
                xw_add(bank, tt)

            if G_FIRST:
                JC_ORDER = list(range(24, 32)) + list(range(0, 24))
            else:
                JC_ORDER = list(range(0, 16)) + list(range(24, 32)) + \
                    list(range(16, 24))

            for t in range(steps):
                cur = pact[t % 3]
                hT = hT0 if t == 0 else yring[:, (t - 1) % (2 * TB)]

                # -- scores at q=(n,m) partitions: psc2[q, h, n'] =
                #    sum_k A[k, q] h[k, n'].  Output free dim is only n'
                #    (16), so the 16 matmuls cost ~7ns each instead of a
                #    [16, 256]-wide stream.
                psc2 = scp.tile([128, 2, NL], F32, tag="psc2")
                for kc in range(KC):
                    for h in range(2):
                        nc.tensor.matmul(
                            psc2[:, h, :],
                            at4[:, kc, 8 * h:8 * h + 8, :].rearrange(
                                "p n m -> p (n m)"),
                            hT[:, kc, :],
                            start=(kc == 0 and h == 0),
                            stop=(kc == KC - 1),
                            skip_group_check=True)

                # -- diag-extract scores per partition q: mask-multiply,
                #    then reduce the n' axis (innermost), keeping h.
                #    (TensorTensorReduce would fuse these but wedges the
                #    device under this runtime.)
                nc.vector.tensor_tensor(
                    junkq[:], psc2[:], maskD[:], op=Alu.mult)
                nc.vector.tensor_reduce(
                    scq[:], junkq[:], axis=mybir.AxisListType.X, op=Alu.add)
                # exp(x) = 1/sigmoid(-x) - 1: keeps every activation in
                # the 'sigmoid_and_others' HW table (sigmoid+tanh), so no
                # 1283ns act-table reload is needed anywhere in the loop.
                # |score*scale| <= ~3, so sigmoid never saturates and the
                # r-1 cancellation only affects negligibly small weights.
                nc.scalar.activation(
                    rrq[:], scq[:], Act.Sigmoid, scale=-SCALE)
                nc.vector.reciprocal(scq[:], rrq[:])
                nc.vector.tensor_scalar(
                    uq[:], scq[:], -1.0, 0.0, op0=Alu.add, op1=Alu.add)

                # -- chain-critical tiny matmuls sit BEFORE the bulk Wh
                # block in the PE stream: they park in the wait queue and
                # win the engine as soon as their DVE inputs land instead
                # of draining behind 1.8us of Wh matmuls.
                pss = scp.tile([8, 2], F32, tag="pss")
                for h in range(2):
                    nc.tensor.matmul(
                        pss[:, h:h + 1], qsel[:], uq[:, h:h + 1],
                        start=(h == 0), stop=(h == 1),
                        skip_group_check=True)
                # unnormalized wdT (runs on DVE during the pss round
                # trip); the 1/sum lands with one final multiply
                nc.vector.tensor_tensor(
                    wdTun[:], maskD[:],
                    uq[:].unsqueeze(2).broadcast_to([128, 2, NL]),
                    op=Alu.mult)
                with nc.allow_low_precision(
                        reason="1/softmax-sum feeds a bf16 matmul anyway"):
                    nc.vector.reciprocal(rsum2[:], pss[:])
                rsb = scp.tile([128, 2], F32, tag="rsb")
                for h in range(2):
                    nc.tensor.matmul(
                        rsb[:, h:h + 1], qselT[:], rsum2[:, h:h + 1],
                        start=(h == 0), stop=(h == 1),
                        skip_group_check=True)
                nc.vector.tensor_tensor(
                    wdT[:], wdTun[:],
                    rsb[:].unsqueeze(2).broadcast_to([128, 2, NL]),
                    op=Alu.mult)

                # -- Wh gate matmuls: emitted AFTER the chain matmuls so
                # those never queue behind them, but BEFORE attn so the
                # gate psum closes with attn; they execute on PE during
                # the softmax DVE/ACT chain.
                for jc in range(JC):
                    for kc in range(KC):
                        nc.tensor.matmul(
                            cur_slice(cur, jc),
                            wh[:, kc, jc * 128:(jc + 1) * 128],
                            hT[:, kc, :], start=False, stop=False,
                            skip_group_check=True)
                if t + 2 < steps:
                    nxt = alloc_bank()
                    pact[(t + 2) % 3] = nxt
                    xw_add(nxt, t + 2)

                # -- attention gate matmuls close each jc's psum group
                # (o-gate jc 16..23 last so its ACT is off the c-path)
                for jc in JC_ORDER:
                    for h in range(2):
                        nc.tensor.matmul(
                            cur_slice(cur, jc),
                            aw2[:, h, jc, :], wdT[:, h, :],
                            start=False, stop=(h == 1),
                            skip_group_check=True)

                # -- gate activations: tanh(g) first (its attn matmuls
                # close first), then ONE sigmoid over the contiguous
                # i/f block, then o last (off the c-path).
                nc.scalar.activation(g_g[:], cur[:, 3], Act.Tanh)
                nc.scalar.activation(g_if[:], cur[:, 0:2], Act.Sigmoid)
                nc.scalar.activation(g_o[:], cur[:, 2], Act.Sigmoid)

                # -- c = f*c + i*g ; h = o*tanh(c)
                fc = smls.tile([128, KC, NL], F32, tag="fc")
                nc.vector.tensor_tensor(
                    fc[:], g_if[:, 1], c_sb[:], op=Alu.mult)
                ig = smls.tile([128, KC, NL], F32, tag="ig")
                nc.vector.tensor_tensor(
                    ig[:], g_if[:, 0], g_g[:], op=Alu.mult)
                nc.vector.tensor_tensor(c_sb[:], fc[:], ig[:], op=Alu.add)
                nc.scalar.activation(th[:], c_sb[:], Act.Tanh)
                hout = yring[:, t % (2 * TB)]
                for q in range(4):
                    nc.vector.tensor_tensor(
                        hout[:, 2 * q:2 * q + 2], g_o[:, 2 * q:2 * q + 2],
                        th[:, 2 * q:2 * q + 2], op=Alu.mult)

                # -- batched y store + xw prefetch every TB steps
                if t % TB == TB - 1:
                    blk = t // TB
                    nc.sync.dma_start(
                        out=d_y[:, blk * TB:(blk + 1) * TB, :, :],
                        in_=yring[:, (blk % 2) * TB:(blk % 2) * TB + TB])
                    if (blk + 2) * TB < steps + TB - 1 and (blk + 2) * TB < T:
                        nc.sync.dma_start(
                            out=xwr[:, blk % 2],
                            in_=d_xw[:, :, (blk + 2) * TB:(blk + 3) * TB, :])
                if t == steps - 1 and steps % TB != 0:
                    blk = t // TB
                    nc.sync.dma_start(
                        out=d_y[:, blk * TB:blk * TB + (steps % TB), :, :],
                        in_=yring[:, (blk % 2) * TB:
                                  (blk % 2) * TB + (steps % TB)])

        _st.close()

    nc.compile()
    _cache[key] = nc
    return nc


def _prepare(x, A, Wx, Wh, Wattn, b):
    x = np.asarray(x, dtype=np.float32)
    A = np.ascontiguousarray(np.asarray(A, dtype=np.float32))
    Wxb = np.ascontiguousarray(
        np.asarray(Wx, dtype=np.float32).astype(np.float16))
    Whb = np.ascontiguousarray(
        np.asarray(Wh, dtype=np.float32).astype(np.float16))
    Wab = np.ascontiguousarray(
        np.asarray(Wattn, dtype=np.float32).astype(np.float16))
    bT = np.ascontiguousarray(
        np.asarray(b, dtype=np.float32).reshape(JC, 128).T)

    id128 = np.eye(128, dtype=np.float16)
    # qsel[q, g] = 1 iff q // 16 == g  (sums the 16 m-positions of local
    # sample-group g); qselT is its transpose (broadcast back to q)
    qsel = np.zeros((128, 8), dtype=np.float16)
    for q in range(128):
        qsel[q, q // 16] = 1.0
    qselT = np.ascontiguousarray(qsel.T)
    # maskD[q, h, n'] = 1 iff n' == 8h + q // 16
    maskD = np.zeros((128, 2, NL), dtype=np.float16)
    for q in range(128):
        for hh in range(2):
            maskD[q, hh, 8 * hh + q // 16] = 1.0
    maskD = maskD.reshape(128, 2 * NL)

    in_maps = []
    for k in range(NCORES):
        xs = x[k * NL:(k + 1) * NL]                      # [16, 64, 1024]
        xT = np.ascontiguousarray(
            xs.transpose(1, 0, 2).reshape(T * NL, D).T.astype(np.float16))
        Ak = np.ascontiguousarray(A[k * NL:(k + 1) * NL].reshape(NL, H, M))
        in_maps.append({
            "xT": xT, "A": Ak, "Wx": Wxb, "Wh": Whb, "Wa": Wab,
            "bT": bT, "id128": id128, "qsel": qsel, "qselT": qselT,
            "maskD": maskD,
        })

    _cache["in_maps"] = in_maps
    return in_maps


def kernel(x, A, Wx, Wh, Wattn, b):
    nc = _build()
    in_maps = _prepare(x, A, Wx, Wh, Wattn, b)
    res = run_bass_kernel_spmd(nc, in_maps, core_ids=list(range(NCORES)))
    outs = []
    for k in range(NCORES):
        yk = res.results[k]["y"]                          # [128, T, KC, NL]
        outs.append(yk.transpose(3, 1, 2, 0).reshape(NL, T, H))
    return np.concatenate(outs, axis=0).astype(np.float32)



# revision 42
# speedup vs baseline: 1.2102x; 1.1500x over previous
"""Attention-LSTM (CaptioningRNN) Trainium2 kernel, v2.

Data-parallel over N=128 across 8 cores (16 samples each), with every
per-step tensor kept TRANSPOSED (feature dim on the 128 partitions, the
16 local samples on the free dim) so each matmul's output free size is
16 and the gate GEMMs run weight-stationary:

  act^T[j, n] = sum_k Wh[k, j] h^T[k, n]          (256 matmuls, ap16)
              + sum_(n',m) AW2[(n',m), j] wdT[(n',m), n]   (64 matmuls)
              + xW^T[j, (t, n)]                   (1 identity matmul)

where AW2[(n,m), j] = sum_h A[n, h, m] Wattn[h, j] is precomputed once
(phase 1), which turns the whole attention-apply + attn@Wattn step into
a real 256-deep contraction: wdT[(n',m), n] = softmax_w[n, m] * (n'==n)
is built by a tiny scatter-matmul from the softmax weights.

Phase 1 also computes xW^T = (x @ Wx + b)^T into a DRAM scratch in a
layout that lets each step load its slice with large descriptors.

h/c state lives as [128 (h mod 128), 8 (h chunk), 16 (n)]; h^T is
written directly into the y-store ring buffer that both the next step's
matmuls and the (batched) y DMA read.
"""

import sys
from contextlib import ExitStack

sys.path.insert(0, "/opt/trn_rl_repo")

import numpy as np

import concourse.bass as bass  # noqa: F401
import concourse.mybir as mybir
import concourse.tile as tile
from concourse import bacc
from concourse.bass_utils import run_bass_kernel_spmd

F32 = mybir.dt.float32
BF16 = mybir.dt.float16  # IEEE fp16: same PE rate as bf16, more mantissa

N, T, D, H = 128, 64, 1024, 1024
K4 = 4 * H            # 4096
NCORES = 8
NL = N // NCORES      # 16 samples per core
M = 16                # spatial positions (4x4)
KC = H // 128         # 8 contraction chunks
JC = K4 // 128        # 32 output chunks
SCALE = 1.0 / float(np.sqrt(H))
TB = 4                # steps per y-store block
XB = 8                # steps per jit-produced xW block
N_WARM = 16
WARM_EN = False
GATE_MODE = 'four'
G_FIRST = True
EXP_ACCUM = True
GATE_COPY = False

Alu = mybir.AluOpType
Act = mybir.ActivationFunctionType

_cache = {}


def _build(steps=T):
    key = ("nc", steps)
    if key in _cache:
        return _cache[key]

    nc = bacc.Bacc("TRN2", target_bir_lowering=False)

    # ---- kernel I/O ----------------------------------------------------
    d_xT = nc.dram_tensor("xT", [D, T * NL], BF16, kind="ExternalInput")
    d_A = nc.dram_tensor("A", [NL, H, M], F32, kind="ExternalInput")
    d_Wx = nc.dram_tensor("Wx", [D, K4], BF16, kind="ExternalInput")
    d_Wh = nc.dram_tensor("Wh", [H, K4], BF16, kind="ExternalInput")
    d_Wa = nc.dram_tensor("Wa", [H, K4], BF16, kind="ExternalInput")
    d_bT = nc.dram_tensor("bT", [128, JC], F32, kind="ExternalInput")
    d_id = nc.dram_tensor("id128", [128, 128], BF16, kind="ExternalInput")
    d_qsel = nc.dram_tensor("qsel", [128, 8], BF16, kind="ExternalInput")
    d_qselT = nc.dram_tensor("qselT", [8, 128], BF16, kind="ExternalInput")
    d_mD = nc.dram_tensor("maskD", [128, 2 * 16], BF16, kind="ExternalInput")
    # outputs
    d_y = nc.dram_tensor("y", [128, T, KC, NL], BF16, kind="ExternalOutput")

    with tile.TileContext(nc) as tc:
      with (
          tc.tile_pool(name="wts", bufs=1) as wts,    # persistent weights
          tc.tile_pool(name="stt", bufs=1) as stt,    # persistent state
      ):
        # ---------------- persistent SBUF tensors -----------------------
        at4 = wts.tile([128, KC, NL, M], BF16, tag="at4")
        aw2 = wts.tile([128, 2, JC, 128], BF16, tag="aw2")
        id128 = wts.tile([128, 128], BF16, tag="id128")
        qsel = wts.tile([128, 8], BF16, tag="qsel")
        qselT = wts.tile([8, 128], BF16, tag="qselT")
        maskD = wts.tile([128, 2, NL], BF16, tag="maskD")
        bT = wts.tile([128, JC], F32, tag="bT")

        c_sb = stt.tile([128, KC, NL], F32, tag="c")
        hT0 = stt.tile([128, KC, NL], BF16, tag="hT0")
        # y ring: 2 buffers x TB steps; h^T of step t lives in
        # yring[t % 2TB]; matmuls of step t+1 read it, y DMA stores it.
        yring = stt.tile([128, 2 * TB, KC, NL], BF16, tag="yring")
        g_if = stt.tile([128, 2, KC, NL], F32, tag="g_if")
        g_o = stt.tile([128, KC, NL], F32, tag="g_o")
        g_g = stt.tile([128, KC, NL], F32, tag="g_g")
        th = stt.tile([128, KC, NL], F32, tag="th")
        # softmax state, all at q=(n,m) partitions (2 h-halves)
        junkq = stt.tile([128, 2, NL], F32, tag="junkq")
        scq = stt.tile([128, 2], F32, tag="scq")
        rrq = stt.tile([128, 2], F32, tag="rrq")
        uq = stt.tile([128, 2], BF16, tag="uq")
        rsum2 = stt.tile([8, 2], BF16, tag="rsum2")
        wdTun = stt.tile([128, 2, NL], BF16, tag="wdTun")
        wdT = stt.tile([128, 2, NL], BF16, tag="wdT")
        # xW^T is produced just-in-time, XB steps per block, 2 blocks
        # ahead of consumption, into a 3-deep SBUF ring (no DRAM spill)
        xwr = stt.tile([128, 3, JC, XB, NL], BF16, tag="xwr")
        # Wx and x^T stay resident so xW blocks can be produced all run
        wx = wts.tile([128, KC, K4], BF16, tag="wx")
        xt = wts.tile([128, KC, T * NL], BF16, tag="xt")


        # ====== Prologue: AW GEMM + first two xW blocks =================
        # DMA issue order = consumption order: A (h0/c0/at4), Wa slices
        # (AW GEMM chases them), xt + Wx slices (block 0/1 production
        # chases), then Wh (first consumed by step 0's gate matmuls,
        # which park in the wait queue until it lands).
        _st = ExitStack()
        whp = _st.enter_context(tc.tile_pool(name="whp", bufs=1))
        wh = whp.tile([128, KC, K4], BF16, tag="wh")
        p1a = _st.enter_context(tc.tile_pool(name="p1a", bufs=8))
        xwp = _st.enter_context(tc.tile_pool(name="xwp", bufs=4,
                                             space="PSUM"))

        def produce_xw(blk, jcs):
            """Emit xW^T production for steps [blk*XB, blk*XB+XB) of
            columns jcs; psum -> xwr ring with the bias folded into the
            required evacuation op (ACT, Identity)."""
            for jc in jcs:
                pxw = xwp.tile([128, XB * NL], F32, tag="pxw", name="pxw")
                for kc in range(KC):
                    nc.tensor.matmul(
                        pxw[:], wx[:, kc, jc * 128:(jc + 1) * 128],
                        xt[:, kc, blk * XB * NL:(blk + 1) * XB * NL],
                        start=(kc == 0), stop=(kc == KC - 1))
                yield pxw, jc

        def evac_xw(blk, items):
            for pxw, jc in items:
                nc.scalar.activation(
                    xwr[:, blk % 3, jc].rearrange("p t n -> p (t n)"),
                    pxw[:], Act.Identity, bias=bT[:, jc:jc + 1])

        if True:
            d_xT_r = d_xT.rearrange("(kc p) r -> p kc r", p=128)
            d_Wx_r = d_Wx.rearrange("(kc p) f -> p kc f", p=128)
            a_sts = []
            for kc in range(KC):
                a_st = p1a.tile([128, NL, M], F32, tag="a_st")
                nc.sync.dma_start(
                    out=a_st[:],
                    in_=d_A.rearrange("n (kc p) m -> kc p n m", p=128)[kc])
                a_sts.append(a_st)
            # small-const loads
            nc.sync.dma_start(out=id128[:], in_=d_id[:])
            nc.sync.dma_start(out=qsel[:], in_=d_qsel[:])
            nc.sync.dma_start(out=qselT[:], in_=d_qselT[:])
            nc.sync.dma_start(
                out=maskD[:], in_=d_mD.rearrange("p (h n) -> p h n", h=2))
            nc.sync.dma_start(out=bT[:], in_=d_bT[:])

            with (
                tc.tile_pool(name="p1bp", bufs=3, space="PSUM") as p1bp,
            ):
                # Wa slices stage INTO wh's slots: the AW GEMM consumes
                # each slice before the (later-issued) Wh DMA overwrites
                # it — the WAR dependency pipelines the two loads.
                d_Wa_r = d_Wa.rearrange("(kc p) f -> p kc f", p=128)
                for jb in range(8):
                    nc.sync.dma_start(
                        out=wh[:, :, jb * 512:(jb + 1) * 512],
                        in_=d_Wa_r[:, :, jb * 512:(jb + 1) * 512])
                nc.sync.dma_start(out=xt[:], in_=d_xT_r[:])
                for jb in range(8):
                    nc.sync.dma_start(
                        out=wx[:, :, jb * 512:(jb + 1) * 512],
                        in_=d_Wx_r[:, :, jb * 512:(jb + 1) * 512])

                # at4 (bf16) and c0 = mean_m A from the staged chunks
                for kc in range(KC):
                    nc.vector.tensor_copy(at4[:, kc], a_sts[kc][:])
                    nc.vector.tensor_reduce(
                        c_sb[:, kc, :], a_sts[kc][:],
                        axis=mybir.AxisListType.X, op=Alu.add)
                nc.vector.tensor_scalar_mul(hT0[:], c_sb[:], 1.0 / M)
                nc.scalar.mul(c_sb[:], c_sb[:], 1.0 / M)

                # AW GEMM (chases the Wa slice DMAs)
                for jb in range(8):
                    for h in range(2):
                        paw = p1bp.tile([128, 512], F32, tag="paw")
                        for kc in range(KC):
                            nc.tensor.matmul(
                                paw[:],
                                at4[:, kc, 8 * h:8 * h + 8, :].rearrange(
                                    "p n m -> p (n m)"),
                                wh[:, kc, jb * 512:(jb + 1) * 512],
                                start=(kc == 0), stop=(kc == KC - 1))
                        nc.vector.tensor_copy(
                            aw2[:, h, 4 * jb:4 * jb + 4, :].rearrange(
                                "p jc q -> p (jc q)"),
                            paw[:])
                # now the real Wh can land over the consumed Wa slices
                d_Wh_r = d_Wh.rearrange("(kc p) f -> p kc f", p=128)
                for jb in range(8):
                    nc.sync.dma_start(
                        out=wh[:, :, jb * 512:(jb + 1) * 512],
                        in_=d_Wh_r[:, :, jb * 512:(jb + 1) * 512])

            # xW blocks 0 and 1 up front (chasing the Wx slice DMAs);
            # block b+2 is produced during the steps of block b.
            for blk in range(min(2, (steps + XB - 1) // XB)):
                for jb in range(8):
                    evac_xw(blk, produce_xw(blk, range(4 * jb, 4 * jb + 4)))

        # ================== Phase 2: recurrent steps ====================
        actp = _st.enter_context(tc.tile_pool(name="actp", bufs=3, space="PSUM"))
        scp = _st.enter_context(tc.tile_pool(name="scp", bufs=1, space="PSUM"))
        smls = _st.enter_context(tc.tile_pool(name="smls", bufs=2))
        if True:
            def warm_on(lhsT, rhs, k):
                if not WARM_EN:
                    return
                """Low-priority PE filler matmuls that become ready only
                once `lhsT` is written: they bridge PE idle gaps in the
                recurrent chain so the p-state ramp stays hot, yielding
                to any ready real matmul (which carries higher priority).
                """
                p = int(np.prod(lhsT.shape[1:]))
                f = int(np.prod(rhs.shape[1:]))
                for _ in range(k):
                    nc.tensor.matmul(
                        pdum[0:p, 0:f], lhsT, rhs,
                        start=True, stop=True)

            pact = [None, None, None]

            def alloc_bank():
                # all four gates in one 2KB psum bank: [i, f, o, g]
                return actp.tile([128, 4, KC, NL], F32, tag="pact",
                                 name="pact")

            def cur_slice(bank, jc):
                return bank[:, jc // 8, jc % 8, :]

            def xw_add(bank, tt):
                # exactly ONE start=True per psum bank: start marks the
                # whole 2KB zero region pending-zero, so a second start
                # would wipe the first half's data
                for half in range(2):
                    nc.tensor.matmul(
                        bank[:, 2 * half:2 * half + 2],
                        id128[:],
                        xwr[:, tt // XB % 3, 16 * half:16 * half + 16,
                            tt % XB, :],
                        start=(half == 0), stop=False,
                        skip_group_check=True)

            for tt in range(min(2, steps)):
                bank = alloc_bank()
                pact[tt % 3] = bank
                xw_add(bank, tt)

            if G_FIRST:
                JC_ORDER = list(range(24, 32)) + list(range(0, 24))
            else:
                JC_ORDER = list(range(0, 16)) + list(range(24, 32)) + \
                    list(range(16, 24))

            for t in range(steps):
                cur = pact[t % 3]
                hT = hT0 if t == 0 else yring[:, (t - 1) % (2 * TB)]

                # -- jit xW production for block t//XB + 2 (4 jc-slices
                # per step); decodes after attn(t-1) drains, so it runs
                # on the PE during the gate/cell phase of this step.
                pblk = t // XB + 2
                pitems = []
                if pblk * XB < steps:
                    pitems = list(produce_xw(
                        pblk, range(4 * (t % XB), 4 * (t % XB) + 4)))

                # -- scores at q=(n,m) partitions: psc2[q, h, n'] =
                #    sum_k A[k, q] h[k, n'].  Output free dim is only n'
                #    (16), so the 16 matmuls cost ~7ns each instead of a
                #    [16, 256]-wide stream.
                # one psum bank holds scores + softmax-sum + 1/sum-bcast
                # (cols 0:16 / 17 / 16); the serial chain order makes the
                # whole-bank pending-zero marking of each start=True safe.
                pscb = scp.tile([128, 2, NL + 2], F32, tag="pscb")
                for kc in range(KC):
                    for h in range(2):
                        nc.tensor.matmul(
                            pscb[:, h, 0:NL],
                            at4[:, kc, 8 * h:8 * h + 8, :].rearrange(
                                "p n m -> p (n m)"),
                            hT[:, kc, :],
                            start=(kc == 0 and h == 0),
                            stop=(kc == KC - 1),
                            skip_group_check=True)

                # -- diag-extract scores per partition q: mask-multiply,
                #    then reduce the n' axis (innermost), keeping h.
                #    (TensorTensorReduce would fuse these but wedges the
                #    device under this runtime.)
                nc.vector.tensor_tensor(
                    junkq[:], pscb[:, :, 0:NL], maskD[:], op=Alu.mult)
                nc.vector.tensor_reduce(
                    scq[:], junkq[:], axis=mybir.AxisListType.X, op=Alu.add)
                # exp(x) = 1/sigmoid(-x) - 1: keeps every activation in
                # the 'sigmoid_and_others' HW table (sigmoid+tanh), so no
                # 1283ns act-table reload is needed anywhere in the loop.
                # |score*scale| <= ~3, so sigmoid never saturates and the
                # r-1 cancellation only affects negligibly small weights.
                nc.scalar.activation(
                    rrq[:], scq[:], Act.Sigmoid, scale=-SCALE)
                nc.vector.reciprocal(scq[:], rrq[:])
                nc.vector.tensor_scalar(
                    uq[:], scq[:], -1.0, 0.0, op0=Alu.add, op1=Alu.add)

                # -- chain-critical tiny matmuls sit BEFORE the bulk Wh
                # block in the PE stream: they park in the wait queue and
                # win the engine as soon as their DVE inputs land instead
                # of draining behind 1.8us of Wh matmuls.
                for h in range(2):
                    nc.tensor.matmul(
                        pscb[0:8, h, NL + 1:NL + 2], qsel[:],
                        uq[:, h:h + 1],
                        start=(h == 0), stop=(h == 1),
                        skip_group_check=True)
                # unnormalized wdT (runs on DVE during the pss round
                # trip); the 1/sum lands with one final multiply
                nc.vector.tensor_tensor(
                    wdTun[:], maskD[:],
                    uq[:].unsqueeze(2).broadcast_to([128, 2, NL]),
                    op=Alu.mult)
                with nc.allow_low_precision(
                        reason="1/softmax-sum feeds a bf16 matmul anyway"):
                    nc.vector.reciprocal(rsum2[:], pscb[0:8, :, NL + 1])
                for h in range(2):
                    nc.tensor.matmul(
                        pscb[:, h, NL:NL + 1], qselT[:], rsum2[:, h:h + 1],
                        start=(h == 0), stop=(h == 1),
                        skip_group_check=True)
                nc.vector.tensor_tensor(
                    wdT[:], wdTun[:],
                    pscb[:, :, NL:NL + 1].broadcast_to([128, 2, NL]),
                    op=Alu.mult)

                # -- Wh gate matmuls: emitted AFTER the chain matmuls so
                # those never queue behind them, but BEFORE attn so the
                # gate psum closes with attn; they execute on PE during
                # the softmax DVE/ACT chain.
                for jc in range(JC):
                    for kc in range(KC):
                        nc.tensor.matmul(
                            cur_slice(cur, jc),
                            wh[:, kc, jc * 128:(jc + 1) * 128],
                            hT[:, kc, :], start=False, stop=False,
                            skip_group_check=True)
                if t + 2 < steps:
                    nxt = alloc_bank()
                    pact[(t + 2) % 3] = nxt
                    xw_add(nxt, t + 2)

                # -- attention gate matmuls close each jc's psum group
                # (o-gate jc 16..23 last so its ACT is off the c-path)
                for jc in JC_ORDER:
                    for h in range(2):
                        nc.tensor.matmul(
                            cur_slice(cur, jc),
                            aw2[:, h, jc, :], wdT[:, h, :],
                            start=False, stop=(h == 1),
                            skip_group_check=True)

                # -- gate activations: tanh(g) first (its attn matmuls
                # close first), then ONE sigmoid over the contiguous
                # i/f block, then o last (off the c-path).
                nc.scalar.activation(g_g[:], cur[:, 3], Act.Tanh)
                nc.scalar.activation(g_if[:], cur[:, 0:2], Act.Sigmoid)
                nc.scalar.activation(g_o[:], cur[:, 2], Act.Sigmoid)

                # -- c = f*c + i*g ; h = o*tanh(c)
                fc = smls.tile([128, KC, NL], F32, tag="fc")
                nc.vector.tensor_tensor(
                    fc[:], g_if[:, 1], c_sb[:], op=Alu.mult)
                ig = smls.tile([128, KC, NL], F32, tag="ig")
                nc.vector.tensor_tensor(
                    ig[:], g_if[:, 0], g_g[:], op=Alu.mult)
                nc.vector.tensor_tensor(c_sb[:], fc[:], ig[:], op=Alu.add)
                nc.scalar.activation(th[:], c_sb[:], Act.Tanh)
                hout = yring[:, t % (2 * TB)]
                for q in range(4):
                    nc.vector.tensor_tensor(
                        hout[:, 2 * q:2 * q + 2], g_o[:, 2 * q:2 * q + 2],
                        th[:, 2 * q:2 * q + 2], op=Alu.mult)

                # -- evacuate this step's produced xW psums on the ACT
                # engine's idle tail (their psums closed early-step)
                evac_xw(pblk, pitems)

                # -- batched y store every TB steps
                if t % TB == TB - 1:
                    blk = t // TB
                    nc.sync.dma_start(
                        out=d_y[:, blk * TB:(blk + 1) * TB, :, :],
                        in_=yring[:, (blk % 2) * TB:(blk % 2) * TB + TB])
                if t == steps - 1 and steps % TB != 0:
                    blk = t // TB
                    nc.sync.dma_start(
                        out=d_y[:, blk * TB:blk * TB + (steps % TB), :, :],
                        in_=yring[:, (blk % 2) * TB:
                                  (blk % 2) * TB + (steps % TB)])

        _st.close()

    nc.compile()
    _cache[key] = nc
    return nc


def _prepare(x, A, Wx, Wh, Wattn, b):
    x = np.asarray(x, dtype=np.float32)
    A = np.ascontiguousarray(np.asarray(A, dtype=np.float32))
    Wxb = np.ascontiguousarray(
        np.asarray(Wx, dtype=np.float32).astype(np.float16))
    Whb = np.ascontiguousarray(
        np.asarray(Wh, dtype=np.float32).astype(np.float16))
    Wab = np.ascontiguousarray(
        np.asarray(Wattn, dtype=np.float32).astype(np.float16))
    bT = np.ascontiguousarray(
        np.asarray(b, dtype=np.float32).reshape(JC, 128).T)

    id128 = np.eye(128, dtype=np.float16)
    # qsel[q, g] = 1 iff q // 16 == g  (sums the 16 m-positions of local
    # sample-group g); qselT is its transpose (broadcast back to q)
    qsel = np.zeros((128, 8), dtype=np.float16)
    for q in range(128):
        qsel[q, q // 16] = 1.0
    qselT = np.ascontiguousarray(qsel.T)
    # maskD[q, h, n'] = 1 iff n' == 8h + q // 16
    maskD = np.zeros((128, 2, NL), dtype=np.float16)
    for q in range(128):
        for hh in range(2):
            maskD[q, hh, 8 * hh + q // 16] = 1.0
    maskD = maskD.reshape(128, 2 * NL)

    in_maps = []
    for k in range(NCORES):
        xs = x[k * NL:(k + 1) * NL]                      # [16, 64, 1024]
        xT = np.ascontiguousarray(
            xs.transpose(1, 0, 2).reshape(T * NL, D).T.astype(np.float16))
        Ak = np.ascontiguousarray(A[k * NL:(k + 1) * NL].reshape(NL, H, M))
        in_maps.append({
            "xT": xT, "A": Ak, "Wx": Wxb, "Wh": Whb, "Wa": Wab,
            "bT": bT, "id128": id128, "qsel": qsel, "qselT": qselT,
            "maskD": maskD,
        })

    _cache["in_maps"] = in_maps
    return in_maps


def kernel(x, A, Wx, Wh, Wattn, b):
    nc = _build()
    in_maps = _prepare(x, A, Wx, Wh, Wattn, b)
    res = run_bass_kernel_spmd(nc, in_maps, core_ids=list(range(NCORES)))
    outs = []
    for k in range(NCORES):
        yk = res.results[k]["y"]                          # [128, T, KC, NL]
        outs.append(yk.transpose(3, 1, 2, 0).reshape(NL, T, H))
    return np.concatenate(outs, axis=0).astype(np.float32)



# revision 46
# speedup vs baseline: 1.2199x; 1.0080x over previous
"""Attention-LSTM (CaptioningRNN) Trainium2 kernel, v2.

Data-parallel over N=128 across 8 cores (16 samples each), with every
per-step tensor kept TRANSPOSED (feature dim on the 128 partitions, the
16 local samples on the free dim) so each matmul's output free size is
16 and the gate GEMMs run weight-stationary:

  act^T[j, n] = sum_k Wh[k, j] h^T[k, n]          (256 matmuls, ap16)
              + sum_(n',m) AW2[(n',m), j] wdT[(n',m), n]   (64 matmuls)
              + xW^T[j, (t, n)]                   (1 identity matmul)

where AW2[(n,m), j] = sum_h A[n, h, m] Wattn[h, j] is precomputed once
(phase 1), which turns the whole attention-apply + attn@Wattn step into
a real 256-deep contraction: wdT[(n',m), n] = softmax_w[n, m] * (n'==n)
is built by a tiny scatter-matmul from the softmax weights.

Phase 1 also computes xW^T = (x @ Wx + b)^T into a DRAM scratch in a
layout that lets each step load its slice with large descriptors.

h/c state lives as [128 (h mod 128), 8 (h chunk), 16 (n)]; h^T is
written directly into the y-store ring buffer that both the next step's
matmuls and the (batched) y DMA read.
"""

import sys
from contextlib import ExitStack

sys.path.insert(0, "/opt/trn_rl_repo")

import numpy as np

import concourse.bass as bass  # noqa: F401
import concourse.mybir as mybir
import concourse.tile as tile
from concourse import bacc
from concourse.bass_utils import run_bass_kernel_spmd

F32 = mybir.dt.float32
BF16 = mybir.dt.float16  # IEEE fp16: same PE rate as bf16, more mantissa

N, T, D, H = 128, 64, 1024, 1024
K4 = 4 * H            # 4096
NCORES = 8
NL = N // NCORES      # 16 samples per core
M = 16                # spatial positions (4x4)
KC = H // 128         # 8 contraction chunks
JC = K4 // 128        # 32 output chunks
SCALE = 1.0 / float(np.sqrt(H))
TB = 4                # steps per y-store block
XB = 8                # steps per jit-produced xW block
N_WARM = 16
WARM_EN = False
GATE_MODE = 'four'
G_FIRST = True
EXP_ACCUM = True
GATE_COPY = False

Alu = mybir.AluOpType
Act = mybir.ActivationFunctionType

_cache = {}


def _build(steps=T):
    key = ("nc", steps)
    if key in _cache:
        return _cache[key]

    nc = bacc.Bacc("TRN2", target_bir_lowering=False)

    # ---- kernel I/O ----------------------------------------------------
    d_xT = nc.dram_tensor("xT", [D, T * NL], BF16, kind="ExternalInput")
    d_A = nc.dram_tensor("A", [NL, H, M], F32, kind="ExternalInput")
    d_Wx = nc.dram_tensor("Wx", [D, K4], BF16, kind="ExternalInput")
    d_Wh = nc.dram_tensor("Wh", [H, K4], BF16, kind="ExternalInput")
    d_Wa = nc.dram_tensor("Wa", [H, K4], BF16, kind="ExternalInput")
    d_bT = nc.dram_tensor("bT", [128, JC], F32, kind="ExternalInput")
    d_id = nc.dram_tensor("id128", [128, 128], BF16, kind="ExternalInput")
    d_qsel = nc.dram_tensor("qsel", [128, 8], BF16, kind="ExternalInput")
    d_qselT = nc.dram_tensor("qselT", [8, 128], BF16, kind="ExternalInput")
    d_mD = nc.dram_tensor("maskD", [128, 2 * 16], BF16, kind="ExternalInput")
    # outputs
    d_y = nc.dram_tensor("y", [128, T, KC, NL], BF16, kind="ExternalOutput")

    with tile.TileContext(nc) as tc:
      with (
          tc.tile_pool(name="wts", bufs=1) as wts,    # persistent weights
          tc.tile_pool(name="stt", bufs=1) as stt,    # persistent state
      ):
        # ---------------- persistent SBUF tensors -----------------------
        at4 = wts.tile([128, KC, NL, M], BF16, tag="at4")
        aw2 = wts.tile([128, 2, JC, 128], BF16, tag="aw2")
        id128 = wts.tile([128, 128], BF16, tag="id128")
        qsel = wts.tile([128, 8], BF16, tag="qsel")
        qselT = wts.tile([8, 128], BF16, tag="qselT")
        maskD = wts.tile([128, 2, NL], BF16, tag="maskD")
        bT = wts.tile([128, JC], F32, tag="bT")

        c_sb = stt.tile([128, KC, NL], F32, tag="c")
        hT0 = stt.tile([128, KC, NL], BF16, tag="hT0")
        # y ring: 2 buffers x TB steps; h^T of step t lives in
        # yring[t % 2TB]; matmuls of step t+1 read it, y DMA stores it.
        yring = stt.tile([128, 2 * TB, KC, NL], BF16, tag="yring")
        g_ifo = stt.tile([128, 3, KC, NL], F32, tag="g_ifo")
        g_g = stt.tile([128, KC, NL], F32, tag="g_g")
        th = stt.tile([128, KC, NL], F32, tag="th")
        # softmax state, all at q=(n,m) partitions (2 h-halves)
        junkq = stt.tile([128, 2, 8], F32, tag="junkq")
        scq = stt.tile([128, 2], F32, tag="scq")
        rrq = stt.tile([128, 2], F32, tag="rrq")
        uq = stt.tile([128, 2], BF16, tag="uq")
        rsum2 = stt.tile([8, 2], BF16, tag="rsum2")
        wdTun = stt.tile([128, 2, NL], BF16, tag="wdTun")
        wdT = stt.tile([128, 2, NL], BF16, tag="wdT")
        # xW^T is produced just-in-time, XB steps per block, 2 blocks
        # ahead of consumption, into a 3-deep SBUF ring (no DRAM spill)
        xwr = stt.tile([128, 3, JC, XB, NL], BF16, tag="xwr")
        # Wx and x^T stay resident so xW blocks can be produced all run
        wx = wts.tile([128, KC, K4], BF16, tag="wx")
        xt = wts.tile([128, KC, T * NL], BF16, tag="xt")


        # ====== Prologue: AW GEMM + first two xW blocks =================
        # DMA issue order = consumption order: A (h0/c0/at4), Wa slices
        # (AW GEMM chases them), xt + Wx slices (block 0/1 production
        # chases), then Wh (first consumed by step 0's gate matmuls,
        # which park in the wait queue until it lands).
        _st = ExitStack()
        whp = _st.enter_context(tc.tile_pool(name="whp", bufs=1))
        wh = whp.tile([128, KC, K4], BF16, tag="wh")
        p1a = _st.enter_context(tc.tile_pool(name="p1a", bufs=8))
        xwp = _st.enter_context(tc.tile_pool(name="xwp", bufs=4,
                                             space="PSUM"))

        def produce_xw(blk, jcs):
            """Emit xW^T production for steps [blk*XB, blk*XB+XB) of
            columns jcs; psum -> xwr ring with the bias folded into the
            required evacuation op (ACT, Identity)."""
            for jc in jcs:
                pxw = xwp.tile([128, XB * NL], F32, tag="pxw", name="pxw")
                for kc in range(KC):
                    nc.tensor.matmul(
                        pxw[:], wx[:, kc, jc * 128:(jc + 1) * 128],
                        xt[:, kc, blk * XB * NL:(blk + 1) * XB * NL],
                        start=(kc == 0), stop=(kc == KC - 1))
                yield pxw, jc

        def evac_xw(blk, items):
            for pxw, jc in items:
                nc.scalar.activation(
                    xwr[:, blk % 3, jc].rearrange("p t n -> p (t n)"),
                    pxw[:], Act.Identity, bias=bT[:, jc:jc + 1])

        if True:
            d_xT_r = d_xT.rearrange("(kc p) r -> p kc r", p=128)
            d_Wx_r = d_Wx.rearrange("(kc p) f -> p kc f", p=128)
            a_sts = []
            for kc in range(KC):
                a_st = p1a.tile([128, NL, M], F32, tag="a_st")
                nc.sync.dma_start(
                    out=a_st[:],
                    in_=d_A.rearrange("n (kc p) m -> kc p n m", p=128)[kc])
                a_sts.append(a_st)
            # small-const loads
            nc.sync.dma_start(out=id128[:], in_=d_id[:])
            nc.sync.dma_start(out=qsel[:], in_=d_qsel[:])
            nc.sync.dma_start(out=qselT[:], in_=d_qselT[:])
            nc.sync.dma_start(
                out=maskD[:], in_=d_mD.rearrange("p (h n) -> p h n", h=2))
            nc.sync.dma_start(out=bT[:], in_=d_bT[:])

            with (
                tc.tile_pool(name="p1bp", bufs=3, space="PSUM") as p1bp,
            ):
                # Wa slices stage INTO wh's slots: the AW GEMM consumes
                # each slice before the (later-issued) Wh DMA overwrites
                # it — the WAR dependency pipelines the two loads.
                d_Wa_r = d_Wa.rearrange("(kc p) f -> p kc f", p=128)
                for jb in range(8):
                    nc.sync.dma_start(
                        out=wh[:, :, jb * 512:(jb + 1) * 512],
                        in_=d_Wa_r[:, :, jb * 512:(jb + 1) * 512])
                nc.sync.dma_start(out=xt[:], in_=d_xT_r[:])
                for jb in range(8):
                    nc.sync.dma_start(
                        out=wx[:, :, jb * 512:(jb + 1) * 512],
                        in_=d_Wx_r[:, :, jb * 512:(jb + 1) * 512])

                # at4 (bf16) and c0 = mean_m A from the staged chunks
                for kc in range(KC):
                    nc.vector.tensor_copy(at4[:, kc], a_sts[kc][:])
                    nc.vector.tensor_reduce(
                        c_sb[:, kc, :], a_sts[kc][:],
                        axis=mybir.AxisListType.X, op=Alu.add)
                nc.vector.tensor_scalar_mul(hT0[:], c_sb[:], 1.0 / M)
                nc.scalar.mul(c_sb[:], c_sb[:], 1.0 / M)

                # AW GEMM (chases the Wa slice DMAs)
                for jb in range(8):
                    for h in range(2):
                        paw = p1bp.tile([128, 512], F32, tag="paw")
                        for kc in range(KC):
                            nc.tensor.matmul(
                                paw[:],
                                at4[:, kc, 8 * h:8 * h + 8, :].rearrange(
                                    "p n m -> p (n m)"),
                                wh[:, kc, jb * 512:(jb + 1) * 512],
                                start=(kc == 0), stop=(kc == KC - 1))
                        nc.vector.tensor_copy(
                            aw2[:, h, 4 * jb:4 * jb + 4, :].rearrange(
                                "p jc q -> p (jc q)"),
                            paw[:])
                # now the real Wh can land over the consumed Wa slices
                d_Wh_r = d_Wh.rearrange("(kc p) f -> p kc f", p=128)
                for jb in range(8):
                    nc.sync.dma_start(
                        out=wh[:, :, jb * 512:(jb + 1) * 512],
                        in_=d_Wh_r[:, :, jb * 512:(jb + 1) * 512])

            # xW blocks 0 and 1 up front (chasing the Wx slice DMAs);
            # block b+2 is produced during the steps of block b.
            for blk in range(min(2, (steps + XB - 1) // XB)):
                for jb in range(8):
                    evac_xw(blk, produce_xw(blk, range(4 * jb, 4 * jb + 4)))

        # ================== Phase 2: recurrent steps ====================
        actp = _st.enter_context(tc.tile_pool(name="actp", bufs=3, space="PSUM"))
        scp = _st.enter_context(tc.tile_pool(name="scp", bufs=1, space="PSUM"))
        smls = _st.enter_context(tc.tile_pool(name="smls", bufs=2))
        if True:
            def warm_on(lhsT, rhs, k):
                if not WARM_EN:
                    return
                """Low-priority PE filler matmuls that become ready only
                once `lhsT` is written: they bridge PE idle gaps in the
                recurrent chain so the p-state ramp stays hot, yielding
                to any ready real matmul (which carries higher priority).
                """
                p = int(np.prod(lhsT.shape[1:]))
                f = int(np.prod(rhs.shape[1:]))
                for _ in range(k):
                    nc.tensor.matmul(
                        pdum[0:p, 0:f], lhsT, rhs,
                        start=True, stop=True)

            pact = [None, None, None]

            def alloc_bank():
                # all four gates in one 2KB psum bank: [i, f, o, g]
                return actp.tile([128, 4, KC, NL], F32, tag="pact",
                                 name="pact")

            def cur_slice(bank, jc):
                return bank[:, jc // 8, jc % 8, :]

            def xw_add(bank, tt):
                # exactly ONE start=True per psum bank: start marks the
                # whole 2KB zero region pending-zero, so a second start
                # would wipe the first half's data
                for half in range(2):
                    nc.tensor.matmul(
                        bank[:, 2 * half:2 * half + 2],
                        id128[:],
                        xwr[:, tt // XB % 3, 16 * half:16 * half + 16,
                            tt % XB, :],
                        start=(half == 0), stop=False,
                        skip_group_check=True)

            for tt in range(min(2, steps)):
                bank = alloc_bank()
                pact[tt % 3] = bank
                xw_add(bank, tt)

            if G_FIRST:
                JC_ORDER = list(range(24, 32)) + list(range(0, 24))
            else:
                JC_ORDER = list(range(0, 16)) + list(range(24, 32)) + \
                    list(range(16, 24))

            for t in range(steps):
                cur = pact[t % 3]
                hT = hT0 if t == 0 else yring[:, (t - 1) % (2 * TB)]

                # -- jit xW production for block t//XB + 2 (4 jc-slices
                # per step); decodes after attn(t-1) drains, so it runs
                # on the PE during the gate/cell phase of this step.
                pblk = t // XB + 2
                pitems = []
                if pblk * XB < steps:
                    pitems = list(produce_xw(
                        pblk, range(4 * (t % XB), 4 * (t % XB) + 4)))

                # -- scores at q=(n,m) partitions: psc2[q, h, n'] =
                #    sum_k A[k, q] h[k, n'].  Output free dim is only n'
                #    (16), so the 16 matmuls cost ~7ns each instead of a
                #    [16, 256]-wide stream.
                # one psum bank holds scores + softmax-sum + 1/sum-bcast
                # (cols 0:8 / 9 / 8); the serial chain order makes the
                # whole-bank pending-zero marking of each start=True safe.
                # Scores for half h only need that half's 8 samples as
                # the rhs, so the diag-extract below is 8 wide, and the
                # diag mask is exactly qsel.
                pscb = scp.tile([128, 2, 10], F32, tag="pscb")
                for kc in range(KC):
                    for h in range(2):
                        nc.tensor.matmul(
                            pscb[:, h, 0:8],
                            at4[:, kc, 8 * h:8 * h + 8, :].rearrange(
                                "p n m -> p (n m)"),
                            hT[:, kc, 8 * h:8 * h + 8],
                            start=(kc == 0 and h == 0),
                            stop=(kc == KC - 1),
                            skip_group_check=True)

                # -- diag-extract scores per partition q: mask-multiply,
                #    then reduce the group axis (innermost), keeping h.
                #    (TensorTensorReduce would fuse these but wedges the
                #    device under this runtime.)
                nc.vector.tensor_tensor(
                    junkq[:], pscb[:, :, 0:8],
                    qsel[:].unsqueeze(1).broadcast_to([128, 2, 8]),
                    op=Alu.mult)
                nc.vector.tensor_reduce(
                    scq[:], junkq[:], axis=mybir.AxisListType.X, op=Alu.add)
                # exp(x) = 1/sigmoid(-x) - 1: keeps every activation in
                # the 'sigmoid_and_others' HW table (sigmoid+tanh), so no
                # 1283ns act-table reload is needed anywhere in the loop.
                # |score*scale| <= ~3, so sigmoid never saturates and the
                # r-1 cancellation only affects negligibly small weights.
                nc.scalar.activation(
                    rrq[:], scq[:], Act.Sigmoid, scale=-SCALE)
                nc.vector.reciprocal(scq[:], rrq[:])
                nc.vector.tensor_scalar(
                    uq[:], scq[:], -1.0, 0.0, op0=Alu.add, op1=Alu.add)

                # -- chain-critical tiny matmuls sit BEFORE the bulk Wh
                # block in the PE stream: they park in the wait queue and
                # win the engine as soon as their DVE inputs land instead
                # of draining behind 1.8us of Wh matmuls.
                for h in range(2):
                    nc.tensor.matmul(
                        pscb[0:8, h, 9:10], qsel[:],
                        uq[:, h:h + 1],
                        start=(h == 0), stop=(h == 1),
                        skip_group_check=True)
                # unnormalized wdT (runs on DVE during the pss round
                # trip); the 1/sum lands with one final multiply
                nc.vector.tensor_tensor(
                    wdTun[:], maskD[:],
                    uq[:].unsqueeze(2).broadcast_to([128, 2, NL]),
                    op=Alu.mult)
                with nc.allow_low_precision(
                        reason="1/softmax-sum feeds a bf16 matmul anyway"):
                    nc.vector.reciprocal(rsum2[:], pscb[0:8, :, 9])
                for h in range(2):
                    nc.tensor.matmul(
                        pscb[:, h, 8:9], qselT[:], rsum2[:, h:h + 1],
                        start=(h == 0), stop=(h == 1),
                        skip_group_check=True)
                nc.vector.tensor_tensor(
                    wdT[:], wdTun[:],
                    pscb[:, :, 8:9].broadcast_to([128, 2, NL]),
                    op=Alu.mult)

                # -- Wh gate matmuls: emitted AFTER the chain matmuls so
                # those never queue behind them, but BEFORE attn so the
                # gate psum closes with attn; they execute on PE during
                # the softmax DVE/ACT chain.
                for jc in range(JC):
                    for kc in range(KC):
                        nc.tensor.matmul(
                            cur_slice(cur, jc),
                            wh[:, kc, jc * 128:(jc + 1) * 128],
                            hT[:, kc, :], start=False, stop=False,
                            skip_group_check=True)
                if t + 2 < steps:
                    nxt = alloc_bank()
                    pact[(t + 2) % 3] = nxt
                    xw_add(nxt, t + 2)

                # -- attention gate matmuls close each jc's psum group
                # (o-gate jc 16..23 last so its ACT is off the c-path)
                for jc in JC_ORDER:
                    for h in range(2):
                        nc.tensor.matmul(
                            cur_slice(cur, jc),
                            aw2[:, h, jc, :], wdT[:, h, :],
                            start=False, stop=(h == 1),
                            skip_group_check=True)

                # -- gate activations: tanh(g) first (its attn matmuls
                # close first), then ONE sigmoid over the contiguous
                # i/f/o block — consecutive ACTs serialize at +219ns in
                # this model, so fewer, bigger ACTs win.
                nc.scalar.activation(g_g[:], cur[:, 3], Act.Tanh)
                nc.scalar.activation(g_ifo[:], cur[:, 0:3], Act.Sigmoid)

                # -- c = f*c + i*g ; h = o*tanh(c)
                fc = smls.tile([128, KC, NL], F32, tag="fc")
                nc.vector.tensor_tensor(
                    fc[:], g_ifo[:, 1], c_sb[:], op=Alu.mult)
                ig = smls.tile([128, KC, NL], F32, tag="ig")
                nc.vector.tensor_tensor(
                    ig[:], g_ifo[:, 0], g_g[:], op=Alu.mult)
                nc.vector.tensor_tensor(c_sb[:], fc[:], ig[:], op=Alu.add)
                nc.scalar.activation(th[:], c_sb[:], Act.Tanh)
                hout = yring[:, t % (2 * TB)]
                for q in range(2):
                    nc.vector.tensor_tensor(
                        hout[:, 4 * q:4 * q + 4], g_ifo[:, 2, 4 * q:4 * q + 4],
                        th[:, 4 * q:4 * q + 4], op=Alu.mult)

                # -- evacuate this step's produced xW psums on the ACT
                # engine's idle tail (their psums closed early-step)
                evac_xw(pblk, pitems)

                # -- batched y store every TB steps
                if t % TB == TB - 1:
                    blk = t // TB
                    nc.sync.dma_start(
                        out=d_y[:, blk * TB:(blk + 1) * TB, :, :],
                        in_=yring[:, (blk % 2) * TB:(blk % 2) * TB + TB])
                if t == steps - 1 and steps % TB != 0:
                    blk = t // TB
                    nc.sync.dma_start(
                        out=d_y[:, blk * TB:blk * TB + (steps % TB), :, :],
                        in_=yring[:, (blk % 2) * TB:
                                  (blk % 2) * TB + (steps % TB)])

        _st.close()

    nc.compile()
    _cache[key] = nc
    return nc


def _prepare(x, A, Wx, Wh, Wattn, b):
    x = np.asarray(x, dtype=np.float32)
    A = np.ascontiguousarray(np.asarray(A, dtype=np.float32))
    Wxb = np.ascontiguousarray(
        np.asarray(Wx, dtype=np.float32).astype(np.float16))
    Whb = np.ascontiguousarray(
        np.asarray(Wh, dtype=np.float32).astype(np.float16))
    Wab = np.ascontiguousarray(
        np.asarray(Wattn, dtype=np.float32).astype(np.float16))
    bT = np.ascontiguousarray(
        np.asarray(b, dtype=np.float32).reshape(JC, 128).T)

    id128 = np.eye(128, dtype=np.float16)
    # qsel[q, g] = 1 iff q // 16 == g  (sums the 16 m-positions of local
    # sample-group g); qselT is its transpose (broadcast back to q)
    qsel = np.zeros((128, 8), dtype=np.float16)
    for q in range(128):
        qsel[q, q // 16] = 1.0
    qselT = np.ascontiguousarray(qsel.T)
    # maskD[q, h, n'] = 1 iff n' == 8h + q // 16
    maskD = np.zeros((128, 2, NL), dtype=np.float16)
    for q in range(128):
        for hh in range(2):
            maskD[q, hh, 8 * hh + q // 16] = 1.0
    maskD = maskD.reshape(128, 2 * NL)

    in_maps = []
    for k in range(NCORES):
        xs = x[k * NL:(k + 1) * NL]                      # [16, 64, 1024]
        xT = np.ascontiguousarray(
            xs.transpose(1, 0, 2).reshape(T * NL, D).T.astype(np.float16))
        Ak = np.ascontiguousarray(A[k * NL:(k + 1) * NL].reshape(NL, H, M))
        in_maps.append({
            "xT": xT, "A": Ak, "Wx": Wxb, "Wh": Whb, "Wa": Wab,
            "bT": bT, "id128": id128, "qsel": qsel, "qselT": qselT,
            "maskD": maskD,
        })

    _cache["in_maps"] = in_maps
    return in_maps


def kernel(x, A, Wx, Wh, Wattn, b):
    nc = _build()
    in_maps = _prepare(x, A, Wx, Wh, Wattn, b)
    res = run_bass_kernel_spmd(nc, in_maps, core_ids=list(range(NCORES)))
    outs = []
    for k in range(NCORES):
        yk = res.results[k]["y"]                          # [128, T, KC, NL]
        outs.append(yk.transpose(3, 1, 2, 0).reshape(NL, T, H))
    return np.concatenate(outs, axis=0).astype(np.float32)

